# revision 35
# baseline (speedup 1.0000x reference)
"""Coupled-attention module as a distributed Bass/Tile kernel on 8 TRN2 cores.

Math notes (exact algebra, not approximations):
- The differential-attention scores are constant along the softmax axis, so
  softmax yields exactly uniform 1/S weights: diff_vector collapses to the
  per-batch mean of (y @ dv_w + dv_b), broadcast over sequence. dq/dk are dead.
- The two gating softmaxes run over the sequence axis (dim=1). Terms constant
  along that axis cancel in softmax exactly: d_theta_b and th1 @ d_theta_w[:H]
  (diff branch) and v_gamma_b (van branch) are all dead. This kills the whole
  th1/bias1 chain and the WD_w / d_theta_w[:H] weights.
- Sharding: rows of the flattened (B*S, H) activations, 256 per core; cores
  0-3 own batch 0, 4-7 batch 1. Each core redundantly computes full-batch K/V
  (collective reshards measure slower than the redundant GEMMs on this part).
- Attention head pairs are packed into disjoint PE row groups (K=64 each); the
  two scores matmuls of a pair write disjoint halves of one [128,512] PSUM
  bank and a single exp ACTIVATE covers both heads.
- The two sequence-axis softmax denominators are summed across the 4-core
  batch group with small AllGathers + local adds. The gathers are split into
  trigger (right after the partial sums) and finish (after the filler GEMMs),
  and every DMA needed before a gather resolves is enqueued ahead of it: the
  sync DMA queue is strictly in-order, so a descriptor gated on the collective
  would head-of-line block all later weight loads (measured 26us PE stall).
- Weight loads use one dma_start per tile (the sync queue costs ~600ns per
  entry) and are prefetched in need-order; the order is chosen so weight-pool
  slot recycling (bufs=5) never gates a DMA on a reader that runs later.
- Compute in bf16 with fp32 accumulation; exp/tanh/sigmoid on ACT; identity
  epilogues on DVE to keep ACT for transcendentals.
"""

import numpy as np
import ml_dtypes

import concourse.bass as bass
import concourse.mybir as mybir
import concourse.tile as tile
from concourse import bacc
from concourse.bass_utils import run_bass_kernel_spmd

B, S, H = 2, 1024, 768
NH, DH = 12, 64
P = 128
RV = 256            # rows per core
KC = H // P         # 6 channel chunks
JC = S // P         # 8 sequence chunks
GROUPS = [[0, 1, 2, 3], [4, 5, 6, 7]]
SCALE = 1.0 / 8.0   # 1/sqrt(DH)

bf16 = mybir.dt.bfloat16
f32 = mybir.dt.float32
AF = mybir.ActivationFunctionType
ALU = mybir.AluOpType
nbf16 = ml_dtypes.bfloat16

W768 = ["vq_w", "vk_w", "vv_w", "dv_w", "van_fc_w", "WV_w", "diff_fc_w",
        "diff_fus_w", "van_fus_w", "nf_w", "final_w"]
W1536 = ["d_theta_w", "v_gamma_w", "diff_out_w", "van_out_w"]
BIAS = ["vq_b", "vk_b", "dv_b", "van_fc_b", "diff_fc_b",
        "diff_out_b", "van_out_b", "diff_fus_b", "van_fus_b",
        "nf_b", "final_b"]


def build(has_vvb: bool):
    nc = bacc.Bacc(None, target_bir_lowering=False, debug=False, num_devices=8)

    xT_d = nc.dram_tensor("xT", [H, RV], bf16, kind="ExternalInput")
    yT_d = nc.dram_tensor("yT", [H, S], bf16, kind="ExternalInput")
    wd = {}
    for w in W768:
        wd[w] = nc.dram_tensor(w, [H, H], bf16, kind="ExternalInput")
    for w in W1536:
        wd[w] = nc.dram_tensor(w, [2 * H, H], bf16, kind="ExternalInput")
    wd["gate_w"] = nc.dram_tensor("gate_w", [2 * H, 1], bf16, kind="ExternalInput")
    wd["nf_out_w"] = nc.dram_tensor("nf_out_w", [2 * H, 1], bf16, kind="ExternalInput")
    bd = {}
    for b in BIAS:
        bd[b] = nc.dram_tensor(b, [H], f32, kind="ExternalInput")
    if has_vvb:
        bd["vv_b"] = nc.dram_tensor("vv_b", [H], f32, kind="ExternalInput")
    out_d = nc.dram_tensor("outT", [H, RV], f32, kind="ExternalOutput")

    with tile.TileContext(nc, num_cores=8) as tc:
        with (
            tc.tile_pool(name="wpool", bufs=6) as wp,
            tc.tile_pool(name="wsmall", bufs=2) as wsp,
            tc.tile_pool(name="acts", bufs=1) as ap,
            tc.tile_pool(name="loop", bufs=2) as lp,
            tc.tile_pool(name="psum", bufs=8, space="PSUM") as pp,
            tc.tile_pool(name="dram", bufs=1, space="DRAM") as dp,
        ):
            def wtile(name, half=None, split=False):
                t = wp.tile([P, KC, H], bf16, name=f"w_{name}_{half}", tag="w")
                src = wd[name]
                if half is not None:
                    src = src[half * H:(half + 1) * H, :]
                src = src.rearrange("(kc p) n -> kc p n", p=P)
                for kc in range(KC):
                    nc.sync.dma_start(t[:, kc, :], src[kc])
                return t

            def btile(name):
                t = ap.tile([P, KC], f32, name=f"b_{name}")
                nc.sync.dma_start(t[:], bd[name].rearrange("(c p) -> p c", p=P))
                return t

            def brow(name):
                t = ap.tile([1, H], f32, name=f"br_{name}")
                nc.sync.dma_start(t[:], bd[name].rearrange("(o c) -> o c", o=1))
                return t

            # ---------------- Q projection first: minimal-dependency PE work
            b_vq = btile("vq_b")
            xT = ap.tile([P, KC, RV], bf16, name="xT")
            for kc in range(KC):
                nc.sync.dma_start(xT[:, kc, :], xT_d.rearrange(
                    "(kc p) n -> kc p n", p=P)[kc])
            w_vq = wtile("vq_w")
            b_vk = btile("vk_b")
            w_vk = wtile("vk_w")
            yT = ap.tile([P, KC, S], bf16, name="yT")
            for kc in range(KC):
                nc.sync.dma_start(yT[:, kc, :], yT_d.rearrange(
                    "(kc p) n -> kc p n", p=P)[kc])
            w_vv = wtile("vv_w")

            # warm up the collective stream early: the first real collective
            # otherwise pays an ~24us trigger-start delay.
            dgi = dp.tile([P, 1], f32, name="dgi")
            dgo = dp.tile([4 * P, 1], f32, name="dgo")
            nc.sync.dma_start(dgi[:], b_vq[:, 0:1])
            nc.gpsimd.collective_compute(
                "AllGather", ALU.bypass, replica_groups=GROUPS,
                ins=[dgi[:]], outs=[dgo[:]])

            qT = ap.tile([P, KC, RV], bf16, name="qT")
            for mc in range(KC):
                ps = pp.tile([P, RV], f32, name=f"qps{mc}", tag="big", bufs=3)
                for kc in range(KC):
                    nc.tensor.matmul(ps[:], w_vq[:, kc, mc * P:(mc + 1) * P],
                                     xT[:, kc, :],
                                     start=(kc == 0), stop=(kc == KC - 1))
                nc.vector.tensor_scalar_add(qT[:, mc, :], ps[:],
                                            b_vq[:, mc:mc + 1])

            ones64 = ap.tile([65, 64], bf16, name="ones64")
            nc.vector.memset(ones64[:], 1.0)
            zb = ap.tile([65, RV], f32, name="zb")
            nc.vector.memset(zb[:], 1.0)
            ones128 = ap.tile([1, P], bf16, name="ones128")
            nc.vector.memset(ones128[:], 1.0)

            # ---------------- K/V projections, emitted in two halves so the
            # attention pairs of the first half overlap the second half ------
            kT = ap.tile([P, KC, S], bf16, name="kT")
            v_aug = ap.tile([P, JC, NH, DH + 1], bf16, name="v_aug")
            nc.vector.memset(v_aug[:, :, :, DH:DH + 1], 1.0)

            # Epilogues of k/v projections run on GPSIMD (otherwise idle):
            # the scores/PV matmuls gate on these, and DVE was measured as
            # the attention-phase serializer (17.8us of PE stalls).
            def kproj(mc):
                for sh in range(2):
                    ps = pp.tile([P, 512], f32, name=f"kps{mc}_{sh}",
                                 tag="big", bufs=3)
                    for kc in range(KC):
                        nc.tensor.matmul(
                            ps[:], w_vk[:, kc, mc * P:(mc + 1) * P],
                            yT[:, kc, sh * 512:(sh + 1) * 512],
                            start=(kc == 0), stop=(kc == KC - 1))
                    nc.scalar.activation(
                        kT[:, mc, sh * 512:(sh + 1) * 512], ps[:],
                        AF.Identity, bias=b_vk[:, mc:mc + 1])

            def vproj(cg):
                for jc in range(JC):
                    ps = pp.tile([P, 384], f32, name=f"vps{jc}_{cg}",
                                 tag="big", bufs=3)
                    for kc in range(KC):
                        nc.tensor.matmul(
                            ps[:], yT[:, kc, jc * P:(jc + 1) * P],
                            w_vv[:, kc, cg * 384:(cg + 1) * 384],
                            start=(kc == 0), stop=(kc == KC - 1))
                    nc.vector.tensor_copy(
                        v_aug[:, jc, cg * 6:(cg + 1) * 6, 0:DH],
                        ps[:].rearrange("p (h d) -> p h d", d=DH))

            # ---------------- per-batch chain pieces ------------------------
            yb = ap.tile([P, KC], f32, name="yb")
            for kc in range(KC):
                nc.vector.tensor_reduce(yb[:, kc:kc + 1], yT[:, kc, :],
                                        axis=mybir.AxisListType.X, op=ALU.add)
            ybt = ap.tile([P, KC], bf16, name="ybt")
            nc.vector.tensor_scalar_mul(ybt[:], yb[:], 1.0 / S)

            def vchain_cm(vec_cm, w_t, bias_cm, name):
                # chan-major out [128, 6] f32 = vec @ W + bias, computed
                # directly in chan-major form (stationary = weight chunk,
                # moving = the [128,1] chan-major vector chunk): no DRAM
                # bounce, so the sync DMA queue never blocks on PE progress.
                out = ap.tile([P, KC], f32, name=f"{name}_cm32")
                for mc in range(KC):
                    ps = pp.tile([P, 1], f32, name=f"{name}ps{mc}",
                                 tag="sps", bufs=2)
                    for kc in range(KC):
                        nc.tensor.matmul(ps[:],
                                         w_t[:, kc, mc * P:(mc + 1) * P],
                                         vec_cm[:, kc:kc + 1],
                                         start=(kc == 0), stop=(kc == KC - 1))
                    if bias_cm is not None:
                        nc.vector.tensor_add(out[:, mc:mc + 1], ps[:],
                                             bias_cm[:, mc:mc + 1])
                    else:
                        nc.vector.tensor_copy(out[:, mc:mc + 1], ps[:])
                return out

            # ---------------- attention: pairs pipelined against K/V -------
            w_dv = wtile("dv_w")
            b_dv = btile("dv_b")
            w_do0 = wtile("diff_out_w", half=0)
            b_dout = btile("diff_out_b")
            if has_vvb:
                b_vv = btile("vv_b")
            vanT = ap.tile([P, KC, RV], bf16, name="vanT")
            acc_t2 = ap.tile([P, KC, RV], f32, name="acc_t2")

            def van_partial(kc, w_t, acc, tg):
                # acc += w[kc-chunk].T @ vanT[kc] as soon as the pair-kc
                # attention output lands; accumulate in SBUF f32 on GPSIMD.
                for mc in range(KC):
                    ps = pp.tile([P, RV], f32, name=f"{tg}p{kc}_{mc}",
                                 tag="pv", bufs=3)
                    nc.tensor.matmul(ps[:], w_t[:, kc, mc * P:(mc + 1) * P],
                                     vanT[:, kc, :], start=True, stop=True)
                    if kc == 0:
                        nc.vector.tensor_copy(acc[:, mc, :], ps[:])
                    else:
                        nc.vector.tensor_add(acc[:, mc, :], acc[:, mc, :],
                                             ps[:])

            def pair_tail(hq, pvs):
                # Both heads' softmax denominators batched into ONE DVE
                # reciprocal (cost is per free-element, partitions are
                # parallel lanes: [2,256] costs the same as [1,256]).
                hp = hq
                nc.vector.tensor_copy(zb[0:1, :], pvs[0][DH:DH + 1, :])
                nc.vector.tensor_copy(zb[64:65, :], pvs[1][DH:DH + 1, :])
                invZb = lp.tile([65, RV], bf16, name=f"invZb{hq}", tag="invZb")
                # one DVE op for both heads: lanes (partitions) are parallel,
                # so [65,256] costs the same as [1,256]; rows 1-63 are unused.
                with nc.allow_low_precision(reason="softmax 1/Z feeds bf16 mul"):
                    nc.vector.reciprocal(invZb[:], zb[:])
                for hh in range(2):
                    bc = pp.tile([64, RV], f32, name=f"bc{2 * hq + hh}",
                                 tag="big", bufs=3)
                    nc.tensor.matmul(bc[:], ones64[64 * hh:64 * hh + 1, :],
                                     invZb[64 * hh:64 * hh + 1, :],
                                     start=True, stop=True)
                    bcs = lp.tile([64, RV], bf16, name=f"bcs{2 * hq + hh}",
                                  tag="bcs")
                    nc.vector.tensor_copy(bcs[:], bc[:])
                    nc.vector.tensor_mul(vanT[hh * 64:hh * 64 + 64, hp, :],
                                         pvs[hh][0:DH, :], bcs[:])
                    if has_vvb:
                        nc.vector.tensor_scalar_add(
                            vanT[hh * 64:hh * 64 + 64, hp, :],
                            vanT[hh * 64:hh * 64 + 64, hp, :],
                            b_vv[hh * 64:hh * 64 + 64, hp:hp + 1])

            def pair_block(hp, prev):
                # scores+exp for pair hp, with the PV matmuls of the previous
                # pair interleaved into the same jc loop so the PE never
                # stalls on ACT's exp backlog. Both heads of the pair share
                # one [128,512] PSUM bank -> one exp ACTIVATE per block.
                e = lp.tile([P, JC, 512], bf16, name=f"e{hp}", tag="expT",
                            bufs=3)
                if prev is not None:
                    hq, eq = prev
                    pvs = [pp.tile([DH + 1, RV], f32, name=f"pv{2 * hq + hh}",
                                   tag="pv", bufs=3) for hh in range(2)]
                for jc in range(JC):
                    for hh in range(2):
                        lo = hh * 64
                        sc = pp.tile([P, RV], f32, name=f"sc{hp}_{jc}_{hh}",
                                     tag="big", bufs=3)
                        nc.tensor.matmul(
                            sc[:],
                            kT[lo:lo + 64, hp, jc * P:(jc + 1) * P],
                            qT[lo:lo + 64, hp, :],
                            start=True, stop=True)
                        nc.scalar.activation(
                            e[:, jc, hh * RV:(hh + 1) * RV], sc[:],
                            AF.Exp, scale=SCALE)
                    if prev is not None:
                        for hh in range(2):
                            nc.tensor.matmul(
                                pvs[hh][:], v_aug[:, jc, 2 * hq + hh, :],
                                eq[:, jc, hh * RV:(hh + 1) * RV],
                                start=(jc == 0), stop=(jc == JC - 1))
                if prev is not None:
                    pair_tail(hq, pvs)
                return e

            def last_pv(hq, eq):
                pvs = [pp.tile([DH + 1, RV], f32, name=f"pv{2 * hq + hh}",
                               tag="pv", bufs=3) for hh in range(2)]
                for jc in range(JC):
                    for hh in range(2):
                        nc.tensor.matmul(
                            pvs[hh][:], v_aug[:, jc, 2 * hq + hh, :],
                            eq[:, jc, hh * RV:(hh + 1) * RV],
                            start=(jc == 0), stop=(jc == JC - 1))
                pair_tail(hq, pvs)

            kproj(0)
            kproj(1)
            kproj(2)
            vproj(0)
            e0 = pair_block(0, None)
            m32 = vchain_cm(ybt, w_dv, b_dv, "m")
            m_cm = ap.tile([P, KC], bf16, name="m_cm")
            nc.vector.tensor_copy(m_cm[:], m32[:])
            e1h = pair_block(1, (0, e0))
            e2h = pair_block(2, (1, e1h))
            w_vfc = wtile("van_fc_w")
            b_vfc = btile("van_fc_b")
            van_partial(0, w_vfc, acc_t2, "t2")
            kproj(3)
            kproj(4)
            kproj(5)
            vproj(1)
            # gating-weight prefetch: issued mid-attention in need-order.
            # With bufs=6 the slot gates (6-back readers) all resolve at or
            # before each weight's emission point, so the in-order sync DMA
            # queue never blocks ahead of the z1 trigger.
            w_dth1 = wtile("d_theta_w", half=1)
            w_WV = wtile("WV_w")
            w_vg0 = wtile("v_gamma_w", half=0)
            w_vo0 = wtile("van_out_w", half=0)
            b_vo = btile("van_out_b")
            b_dfc = btile("diff_fc_b")
            ws_gate = wsp.tile([P, 2 * KC, 1], bf16, name="ws_gate", tag="ws")
            nc.sync.dma_start(ws_gate[:], wd["gate_w"].rearrange(
                "(c p) o -> p c o", p=P))
            ws_nf = wsp.tile([P, 2 * KC, 1], bf16, name="ws_nf", tag="ws")
            nc.sync.dma_start(ws_nf[:], wd["nf_out_w"].rearrange(
                "(c p) o -> p c o", p=P))
            bias2 = vchain_cm(m_cm, w_do0, b_dout, "bias2")
            van_partial(1, w_vfc, acc_t2, "t2")
            w_dfc = wtile("diff_fc_w")
            e3h = pair_block(3, (2, e2h))
            van_partial(2, w_vfc, acc_t2, "t2")
            b_dfus = btile("diff_fus_b")
            b_vfus = btile("van_fus_b")
            b_nf = btile("nf_b")
            b_fin = btile("final_b")
            e4h = pair_block(4, (3, e3h))
            van_partial(3, w_vfc, acc_t2, "t2")
            e5h = pair_block(5, (4, e4h))
            van_partial(4, w_vfc, acc_t2, "t2")
            last_pv(5, e5h)
            van_partial(5, w_vfc, acc_t2, "t2")
            # vg1 slot-gates on van_partial(5) (vfc's last read): emit here so
            # the queue unblocks right away; it lands during the e1 gemm.
            w_vg1 = wtile("v_gamma_w", half=1)

            # ---------------- gating network --------------------------------
            def gemm(pairs, func, bias_t=None, accum_t=None, name="g",
                     out_dt=bf16, pre=None):
                out = ap.tile([P, KC, RV], out_dt, name=name)
                nmm = len(pairs) * KC
                for mc in range(KC):
                    ps = pp.tile([P, RV], f32, name=f"{name}ps{mc}", tag="big",
                                 bufs=3)
                    i = 0
                    for wt, at in pairs:
                        for kc in range(KC):
                            nc.tensor.matmul(ps[:],
                                             wt[:, kc, mc * P:(mc + 1) * P],
                                             at[:, kc, :],
                                             start=(i == 0), stop=(i == nmm - 1))
                            i += 1
                    src = ps
                    if pre is not None:
                        tmp = lp.tile([P, RV], f32, name=f"{name}pre{mc}",
                                      tag="pretmp")
                        nc.vector.tensor_add(tmp[:], ps[:], pre[:, mc, :])
                        src = tmp
                    if func == AF.Identity and accum_t is None:
                        if bias_t is not None:
                            nc.vector.tensor_scalar_add(out[:, mc, :], src[:],
                                                        bias_t[:, mc:mc + 1])
                        else:
                            nc.vector.tensor_copy(out[:, mc, :], src[:])
                    else:
                        nc.scalar.activation(
                            out[:, mc, :], src[:], func,
                            bias=(bias_t[:, mc:mc + 1] if bias_t is not None
                                  else 0.0),
                            accum_out=(accum_t[:, mc:mc + 1]
                                       if accum_t is not None else None))
                return out

            def ag_start(part, name):
                gi = dp.tile([P, KC], f32, name=f"gi_{name}")
                go = dp.tile([4 * P, KC], f32, name=f"go_{name}")
                nc.sync.dma_start(gi[:], part[:])
                nc.gpsimd.collective_compute(
                    "AllGather", ALU.bypass, replica_groups=GROUPS,
                    ins=[gi[:]], outs=[go[:]])
                return go

            def ag_finish(go, name):
                zt = ap.tile([P, 4, KC], f32, name=f"zt_{name}")
                nc.sync.dma_start(zt[:], go.rearrange("(r p) c -> p r c", p=P))
                z = ap.tile([P, KC], f32, name=f"z_{name}")
                nc.vector.tensor_add(z[:], zt[:, 0, :], zt[:, 1, :])
                nc.vector.tensor_add(z[:], z[:], zt[:, 2, :])
                nc.vector.tensor_add(z[:], z[:], zt[:, 3, :])
                return z

            theta2 = ap.tile([P, KC, RV], bf16, name="theta2")
            for mc in range(KC):
                nc.scalar.activation(theta2[:, mc, :], acc_t2[:, mc, :],
                                     AF.Tanh, bias=b_vfc[:, mc:mc + 1])

            part1 = ap.tile([P, KC], f32, name="part1")
            e1 = gemm([(w_dth1, theta2)], AF.Exp, accum_t=part1, name="e1")
            go1 = ag_start(part1, "z1")

            # --- AllGather-1 bubble fillers (independent of z1) -------------
            # gamma1 and voa interleaved at the mc level so each one's ACT
            # epilogues hide under the other's matmuls. Weight DMAs for the
            # post-z1 GEMMs are emitted here in need-order; each slot gate
            # resolves no later than the previous one (monotone), so the
            # queue drains without head-of-line blocking.
            w_do1 = wtile("diff_out_w", half=1)
            gamma1 = ap.tile([P, KC, RV], bf16, name="gamma1")
            voa = ap.tile([P, KC, RV], f32, name="voa")
            for mc in range(KC):
                ps1 = pp.tile([P, RV], f32, name=f"g1ps{mc}", tag="big",
                              bufs=3)
                for kc in range(KC):
                    nc.tensor.matmul(ps1[:], w_WV[:, kc, mc * P:(mc + 1) * P],
                                     vanT[:, kc, :],
                                     start=(kc == 0), stop=(kc == KC - 1))
                nc.scalar.activation(gamma1[:, mc, :], ps1[:], AF.Tanh)
                ps2 = pp.tile([P, RV], f32, name=f"voaps{mc}", tag="big",
                              bufs=3)
                for kc in range(KC):
                    nc.tensor.matmul(ps2[:], w_vo0[:, kc, mc * P:(mc + 1) * P],
                                     vanT[:, kc, :],
                                     start=(kc == 0), stop=(kc == KC - 1))
                nc.vector.tensor_scalar_add(voa[:, mc, :], ps2[:],
                                            b_vo[:, mc:mc + 1])
            w_dfus = wtile("diff_fus_w")
            z2a = gemm([(w_vg0, gamma1)], AF.Identity, name="z2a", out_dt=f32)
            w_vo1 = wtile("van_out_w", half=1)
            ps_nf = pp.tile([1, RV], f32, name="nfps", tag="sps", bufs=2)
            for kc in range(KC):
                nc.tensor.matmul(ps_nf[:], ws_nf[:, kc, :], vanT[:, kc, :],
                                 start=(kc == 0), stop=False,
                                 skip_group_check=True)
            w_vfus = wtile("van_fus_w")
            z1 = ag_finish(go1, "z1")

            s1 = ap.tile([P, KC], f32, name="s1")
            nc.vector.reciprocal(s1[:], z1[:])
            nc.vector.tensor_mul(s1[:], s1[:], m32[:])
            dth = ap.tile([P, KC, RV], bf16, name="dth")
            for mc in range(KC):
                nc.vector.tensor_scalar_mul(dth[:, mc, :], e1[:, mc, :],
                                            s1[:, mc:mc + 1])

            gamma2 = gemm([(w_dfc, dth)], AF.Tanh, bias_t=b_dfc, name="gamma2")
            w_nf = wtile("nf_w")
            part2 = ap.tile([P, KC], f32, name="part2")
            e2 = gemm([(w_vg1, gamma2)], AF.Exp, accum_t=part2, pre=z2a,
                      name="e2")
            go2 = ag_start(part2, "z2")
            w_fin = wtile("final_w")

            # --- AllGather-2 bubble fillers --------------------------------
            dout = gemm([(w_do1, dth)], AF.Tanh, bias_t=bias2, name="dout")
            dfus = gemm([(w_dfus, dout)], AF.Tanh, bias_t=b_dfus, name="dfus")
            z2 = ag_finish(go2, "z2")

            s2 = ap.tile([P, KC], f32, name="s2")
            nc.vector.reciprocal(s2[:], z2[:])
            ag = ap.tile([P, KC, RV], bf16, name="ag")
            for mc in range(KC):
                nc.vector.scalar_tensor_tensor(
                    ag[:, mc, :], e2[:, mc, :], s2[:, mc:mc + 1],
                    vanT[:, mc, :], op0=ALU.mult, op1=ALU.mult)

            vout = gemm([(w_vo1, ag)], AF.Tanh, pre=voa, name="vout")
            vfus = gemm([(w_vfus, vout)], AF.Tanh, bias_t=b_vfus, name="vfus")
            diffv = ap.tile([P, KC, RV], bf16, name="diffv")
            for mc in range(KC):
                nc.vector.tensor_sub(diffv[:, mc, :], vfus[:, mc, :],
                                     dfus[:, mc, :])

            # gate (M=1 GEMM over both fusion tensors)
            ps_g = pp.tile([1, RV], f32, name="gateps", tag="sps", bufs=2)
            i = 0
            for at, base in [(dfus, 0), (vfus, KC)]:
                for kc in range(KC):
                    nc.tensor.matmul(ps_g[:], ws_gate[:, base + kc, :],
                                     at[:, kc, :],
                                     start=(i == 0), stop=(i == 2 * KC - 1))
                    i += 1
            gb16 = ap.tile([1, RV], bf16, name="gb16")
            nc.scalar.activation(gb16[:], ps_g[:], AF.Sigmoid)
            gbc = pp.tile([P, RV], f32, name="gbc", tag="pv", bufs=3)
            nc.tensor.matmul(gbc[:], ones128[:], gb16[:], start=True, stop=True)

            fus = ap.tile([P, KC, RV], bf16, name="fus")
            for mc in range(KC):
                t2 = lp.tile([P, RV], bf16, name=f"ft2_{mc}", tag="ft2")
                nc.vector.tensor_mul(t2[:], diffv[:, mc, :], gbc[:])
                nc.vector.tensor_add(fus[:, mc, :], t2[:], dfus[:, mc, :])

            # tnf first, so the nf sigmoid/broadcast is ready before the
            # final tanh GEMM and its fused mul+store epilogue.
            tnf = gemm([(w_nf, fus)], AF.Identity, bias_t=b_nf, name="tnf")
            for kc in range(KC):
                nc.tensor.matmul(ps_nf[:], ws_nf[:, KC + kc, :], tnf[:, kc, :],
                                 start=False, stop=(kc == KC - 1),
                                 skip_group_check=True)
            nb16 = ap.tile([1, RV], bf16, name="nb16")
            nc.scalar.activation(nb16[:], ps_nf[:], AF.Sigmoid)
            nbc = pp.tile([P, RV], f32, name="nbc", tag="pv", bufs=3)
            nc.tensor.matmul(nbc[:], ones128[:], nb16[:], start=True, stop=True)

            od = out_d.rearrange("(mc p) n -> mc p n", p=P)
            for mc in range(KC):
                ps = pp.tile([P, RV], f32, name=f"ftps{mc}", tag="big", bufs=3)
                for kc in range(KC):
                    nc.tensor.matmul(ps[:], w_fin[:, kc, mc * P:(mc + 1) * P],
                                     fus[:, kc, :],
                                     start=(kc == 0), stop=(kc == KC - 1))
                ftc = lp.tile([P, RV], bf16, name=f"ftc{mc}", tag="ftc")
                nc.scalar.activation(ftc[:], ps[:], AF.Tanh,
                                     bias=b_fin[:, mc:mc + 1])
                ot = lp.tile([P, RV], f32, name=f"ot{mc}", tag="ot", bufs=3)
                nc.vector.tensor_mul(ot[:], ftc[:], nbc[:])
                nc.sync.dma_start(od[mc], ot[:])

    nc.compile()
    return nc


_CACHE = {}


def _prep_in_maps(inputs):
    x = np.asarray(inputs["x"], np.float32)
    y = np.asarray(inputs["y"], np.float32)
    has_vvb = bool(np.any(np.asarray(inputs["vv_b"]) != 0))

    xt = np.ascontiguousarray(x.reshape(B * S, H).T).astype(nbf16)   # [H, 2048]
    yts = [np.ascontiguousarray(y[b].T).astype(nbf16) for b in range(B)]

    base = {}
    for w in W768 + W1536 + ["gate_w", "nf_out_w"]:
        base[w] = np.asarray(inputs[w], np.float32).astype(nbf16)
    for b in BIAS:
        base[b] = np.ascontiguousarray(np.asarray(inputs[b], np.float32))
    if has_vvb:
        base["vv_b"] = np.ascontiguousarray(np.asarray(inputs["vv_b"], np.float32))

    in_maps = []
    for c in range(8):
        bat = c // 4
        m = dict(base)
        m["xT"] = np.ascontiguousarray(xt[:, c * RV:(c + 1) * RV])
        m["yT"] = yts[bat]
        in_maps.append(m)
    return in_maps, has_vvb


def kernel(**inputs):
    in_maps, has_vvb = _prep_in_maps(inputs)
    if has_vvb not in _CACHE:
        _CACHE[has_vvb] = build(has_vvb)
    nc = _CACHE[has_vvb]

    res = run_bass_kernel_spmd(nc, in_maps, core_ids=list(range(8)))
    full = np.concatenate([res.results[c]["outT"] for c in range(8)], axis=1)
    return np.ascontiguousarray(full.T.reshape(B, S, H)).astype(np.float32)


if __name__ == "__main__":
    rng = np.random.default_rng(0)
    ins = {"x": rng.standard_normal((B, S, H)).astype(np.float32),
           "y": rng.standard_normal((B, S, H)).astype(np.float32)}
    for w in W768 + W1536 + ["dq_w", "dk_w", "WD_w"]:
        shp = (H, H) if w not in W1536 else (2 * H, H)
        ins[w] = (rng.standard_normal(shp) * 0.02).astype(np.float32)
    ins["gate_w"] = (rng.standard_normal((2 * H, 1)) * 0.02).astype(np.float32)
    ins["nf_out_w"] = (rng.standard_normal((2 * H, 1)) * 0.02).astype(np.float32)
    for b in BIAS + ["vv_b", "dq_b", "dk_b", "d_theta_b", "v_gamma_b"]:
        ins[b] = np.zeros(H, np.float32)
    out = kernel(**ins)
    print("out", out.shape, out.dtype, np.abs(out).mean())


# revision 37
# speedup vs baseline: 1.1220x; 1.1220x over previous
"""Coupled-attention module as a distributed Bass/Tile kernel on 8 TRN2 cores.

Math notes (exact algebra, not approximations):
- The differential-attention scores are constant along the softmax axis, so
  softmax yields exactly uniform 1/S weights: diff_vector collapses to the
  per-batch mean of (y @ dv_w + dv_b), broadcast over sequence. dq/dk are dead.
- The two gating softmaxes run over the sequence axis (dim=1). Terms constant
  along that axis cancel in softmax exactly: d_theta_b and th1 @ d_theta_w[:H]
  (diff branch) and v_gamma_b (van branch) are all dead. This kills the whole
  th1/bias1 chain and the WD_w / d_theta_w[:H] weights.
- Sharding: rows of the flattened (B*S, H) activations, 256 per core; cores
  0-3 own batch 0, 4-7 batch 1. Each core redundantly computes full-batch K/V
  (collective reshards measure slower than the redundant GEMMs on this part).
- Attention head pairs are packed into disjoint PE row groups (K=64 each); the
  two scores matmuls of a pair write disjoint halves of one [128,512] PSUM
  bank and a single exp ACTIVATE covers both heads.
- The two sequence-axis softmax denominators are summed across the 4-core
  batch group with small AllGathers + local adds. The gathers are split into
  trigger (right after the partial sums) and finish (after the filler GEMMs),
  and every DMA needed before a gather resolves is enqueued ahead of it: the
  sync DMA queue is strictly in-order, so a descriptor gated on the collective
  would head-of-line block all later weight loads (measured 26us PE stall).
- Weight loads use one dma_start per tile (the sync queue costs ~600ns per
  entry) and are prefetched in need-order; the order is chosen so weight-pool
  slot recycling (bufs=5) never gates a DMA on a reader that runs later.
- Compute in bf16 with fp32 accumulation; exp/tanh/sigmoid on ACT; identity
  epilogues on DVE to keep ACT for transcendentals.
"""

import numpy as np
import ml_dtypes

import concourse.bass as bass
import concourse.mybir as mybir
import concourse.tile as tile
from concourse import bacc
from concourse.bass_utils import run_bass_kernel_spmd

B, S, H = 2, 1024, 768
NH, DH = 12, 64
P = 128
RV = 256            # rows per core
KC = H // P         # 6 channel chunks
JC = S // P         # 8 sequence chunks
GROUPS = [[0, 1, 2, 3], [4, 5, 6, 7]]
SCALE = 1.0 / 8.0   # 1/sqrt(DH)

bf16 = mybir.dt.bfloat16
f32 = mybir.dt.float32
AF = mybir.ActivationFunctionType
ALU = mybir.AluOpType
nbf16 = ml_dtypes.bfloat16

W768 = ["vq_w", "vk_w", "vv_w", "dv_w", "van_fc_w", "WV_w", "diff_fc_w",
        "diff_fus_w", "van_fus_w", "nf_w", "final_w"]
W1536 = ["d_theta_w", "v_gamma_w", "diff_out_w", "van_out_w"]
BIAS = ["vq_b", "vk_b", "dv_b", "van_fc_b", "diff_fc_b",
        "diff_out_b", "van_out_b", "diff_fus_b", "van_fus_b",
        "nf_b", "final_b"]


def build(has_vvb: bool):
    nc = bacc.Bacc(None, target_bir_lowering=False, debug=False, num_devices=8)

    xT_d = nc.dram_tensor("xT", [H, RV], bf16, kind="ExternalInput")
    yT_d = nc.dram_tensor("yT", [H, S], bf16, kind="ExternalInput")
    wd = {}
    for w in W768:
        wd[w] = nc.dram_tensor(w, [H, H], bf16, kind="ExternalInput")
    for w in W1536:
        wd[w] = nc.dram_tensor(w, [2 * H, H], bf16, kind="ExternalInput")
    wd["gate_w"] = nc.dram_tensor("gate_w", [2 * H, 1], bf16, kind="ExternalInput")
    wd["nf_out_w"] = nc.dram_tensor("nf_out_w", [2 * H, 1], bf16, kind="ExternalInput")
    bd = {}
    for b in BIAS:
        bd[b] = nc.dram_tensor(b, [H], f32, kind="ExternalInput")
    if has_vvb:
        bd["vv_b"] = nc.dram_tensor("vv_b", [H], f32, kind="ExternalInput")
    out_d = nc.dram_tensor("outT", [H, RV], f32, kind="ExternalOutput")

    with tile.TileContext(nc, num_cores=8) as tc:
        with (
            tc.tile_pool(name="wpool", bufs=6) as wp,
            tc.tile_pool(name="wsmall", bufs=2) as wsp,
            tc.tile_pool(name="acts", bufs=1) as ap,
            tc.tile_pool(name="loop", bufs=2) as lp,
            tc.tile_pool(name="psum", bufs=8, space="PSUM") as pp,
            tc.tile_pool(name="dram", bufs=1, space="DRAM") as dp,
        ):
            def wtile(name, half=None, split=False):
                t = wp.tile([P, KC, H], bf16, name=f"w_{name}_{half}", tag="w")
                src = wd[name]
                if half is not None:
                    src = src[half * H:(half + 1) * H, :]
                src = src.rearrange("(kc p) n -> kc p n", p=P)
                for kc in range(KC):
                    nc.sync.dma_start(t[:, kc, :], src[kc])
                return t

            def btile(name):
                t = ap.tile([P, KC], f32, name=f"b_{name}")
                nc.sync.dma_start(t[:], bd[name].rearrange("(c p) -> p c", p=P))
                return t

            def brow(name):
                t = ap.tile([1, H], f32, name=f"br_{name}")
                nc.sync.dma_start(t[:], bd[name].rearrange("(o c) -> o c", o=1))
                return t

            # ---------------- Q projection first: minimal-dependency PE work
            b_vq = btile("vq_b")
            xT = ap.tile([P, KC, RV], bf16, name="xT")
            for kc in range(KC):
                nc.sync.dma_start(xT[:, kc, :], xT_d.rearrange(
                    "(kc p) n -> kc p n", p=P)[kc])
            w_vq = wtile("vq_w")
            b_vk = btile("vk_b")
            w_vk = wtile("vk_w")
            yT = ap.tile([P, KC, S], bf16, name="yT")
            for kc in range(KC):
                nc.sync.dma_start(yT[:, kc, :], yT_d.rearrange(
                    "(kc p) n -> kc p n", p=P)[kc])
            w_vv = wtile("vv_w")

            # warm up the collective stream early: the first real collective
            # otherwise pays an ~24us trigger-start delay.
            dgi = dp.tile([P, 1], f32, name="dgi")
            dgo = dp.tile([4 * P, 1], f32, name="dgo")
            nc.sync.dma_start(dgi[:], b_vq[:, 0:1])
            nc.gpsimd.collective_compute(
                "AllGather", ALU.bypass, replica_groups=GROUPS,
                ins=[dgi[:]], outs=[dgo[:]])

            qT = ap.tile([P, KC, RV], bf16, name="qT")
            for mc in range(KC):
                ps = pp.tile([P, RV], f32, name=f"qps{mc}", tag="big", bufs=3)
                for kc in range(KC):
                    nc.tensor.matmul(ps[:], w_vq[:, kc, mc * P:(mc + 1) * P],
                                     xT[:, kc, :],
                                     start=(kc == 0), stop=(kc == KC - 1))
                nc.vector.tensor_scalar_add(qT[:, mc, :], ps[:],
                                            b_vq[:, mc:mc + 1])

            ones64 = ap.tile([1, 64], bf16, name="ones64")
            nc.vector.memset(ones64[:], 1.0)
            ones128 = ap.tile([1, P], bf16, name="ones128")
            nc.vector.memset(ones128[:], 1.0)

            # ---------------- K/V projections, emitted in two halves so the
            # attention pairs of the first half overlap the second half ------
            kT = ap.tile([P, KC, S], bf16, name="kT")
            v_aug = ap.tile([P, JC, NH, DH + 1], bf16, name="v_aug")
            nc.vector.memset(v_aug[:, :, :, DH:DH + 1], 1.0)

            # Epilogues of k/v projections run on GPSIMD (otherwise idle):
            # the scores/PV matmuls gate on these, and DVE was measured as
            # the attention-phase serializer (17.8us of PE stalls).
            def kproj(mc):
                for sh in range(2):
                    ps = pp.tile([P, 512], f32, name=f"kps{mc}_{sh}",
                                 tag="big", bufs=3)
                    for kc in range(KC):
                        nc.tensor.matmul(
                            ps[:], w_vk[:, kc, mc * P:(mc + 1) * P],
                            yT[:, kc, sh * 512:(sh + 1) * 512],
                            start=(kc == 0), stop=(kc == KC - 1))
                    nc.scalar.activation(
                        kT[:, mc, sh * 512:(sh + 1) * 512], ps[:],
                        AF.Identity, bias=b_vk[:, mc:mc + 1])

            def vproj(cg):
                for jc in range(JC):
                    ps = pp.tile([P, 384], f32, name=f"vps{jc}_{cg}",
                                 tag="big", bufs=3)
                    for kc in range(KC):
                        nc.tensor.matmul(
                            ps[:], yT[:, kc, jc * P:(jc + 1) * P],
                            w_vv[:, kc, cg * 384:(cg + 1) * 384],
                            start=(kc == 0), stop=(kc == KC - 1))
                    nc.vector.tensor_copy(
                        v_aug[:, jc, cg * 6:(cg + 1) * 6, 0:DH],
                        ps[:].rearrange("p (h d) -> p h d", d=DH))

            # ---------------- per-batch chain pieces ------------------------
            yb = ap.tile([P, KC], f32, name="yb")
            for kc in range(KC):
                nc.vector.tensor_reduce(yb[:, kc:kc + 1], yT[:, kc, :],
                                        axis=mybir.AxisListType.X, op=ALU.add)
            ybt = ap.tile([P, KC], bf16, name="ybt")
            nc.vector.tensor_scalar_mul(ybt[:], yb[:], 1.0 / S)

            def vchain_cm(vec_cm, w_t, bias_cm, name):
                # chan-major out [128, 6] f32 = vec @ W + bias, computed
                # directly in chan-major form (stationary = weight chunk,
                # moving = the [128,1] chan-major vector chunk): no DRAM
                # bounce, so the sync DMA queue never blocks on PE progress.
                out = ap.tile([P, KC], f32, name=f"{name}_cm32")
                for mc in range(KC):
                    ps = pp.tile([P, 1], f32, name=f"{name}ps{mc}",
                                 tag="sps", bufs=2)
                    for kc in range(KC):
                        nc.tensor.matmul(ps[:],
                                         w_t[:, kc, mc * P:(mc + 1) * P],
                                         vec_cm[:, kc:kc + 1],
                                         start=(kc == 0), stop=(kc == KC - 1))
                    if bias_cm is not None:
                        nc.vector.tensor_add(out[:, mc:mc + 1], ps[:],
                                             bias_cm[:, mc:mc + 1])
                    else:
                        nc.vector.tensor_copy(out[:, mc:mc + 1], ps[:])
                return out

            # ---------------- attention: pairs pipelined against K/V -------
            w_dv = wtile("dv_w")
            b_dv = btile("dv_b")
            w_do0 = wtile("diff_out_w", half=0)
            b_dout = btile("diff_out_b")
            if has_vvb:
                b_vv = btile("vv_b")
            vanT = ap.tile([P, KC, RV], bf16, name="vanT")
            acc_t2 = ap.tile([P, KC, RV], f32, name="acc_t2")

            def van_partial(kc, w_t, acc, tg):
                # acc += w[kc-chunk].T @ vanT[kc] as soon as the pair-kc
                # attention output lands; accumulate in SBUF f32 on GPSIMD.
                for mc in range(KC):
                    ps = pp.tile([P, RV], f32, name=f"{tg}p{kc}_{mc}",
                                 tag="pv", bufs=3)
                    nc.tensor.matmul(ps[:], w_t[:, kc, mc * P:(mc + 1) * P],
                                     vanT[:, kc, :], start=True, stop=True)
                    if kc == 0:
                        nc.vector.tensor_copy(acc[:, mc, :], ps[:])
                    else:
                        nc.vector.tensor_add(acc[:, mc, :], acc[:, mc, :],
                                             ps[:])

            def pair_tail(hq, pvs):
                hp = hq
                for hh in range(2):
                    invZb = lp.tile([1, RV], bf16, name=f"invZb{2 * hq + hh}",
                                    tag="invZb")
                    with nc.allow_low_precision(reason="1/Z feeds bf16 mul"):
                        nc.vector.reciprocal(invZb[:], pvs[hh][DH:DH + 1, :])
                    bc = pp.tile([64, RV], f32, name=f"bc{2 * hq + hh}",
                                 tag="big", bufs=3)
                    nc.tensor.matmul(bc[:], ones64[:], invZb[:],
                                     start=True, stop=True)
                    bcs = lp.tile([64, RV], bf16, name=f"bcs{2 * hq + hh}",
                                  tag="bcs")
                    nc.vector.tensor_copy(bcs[:], bc[:])
                    nc.vector.tensor_mul(vanT[hh * 64:hh * 64 + 64, hp, :],
                                         pvs[hh][0:DH, :], bcs[:])
                    if has_vvb:
                        nc.vector.tensor_scalar_add(
                            vanT[hh * 64:hh * 64 + 64, hp, :],
                            vanT[hh * 64:hh * 64 + 64, hp, :],
                            b_vv[hh * 64:hh * 64 + 64, hp:hp + 1])

            def pair_block(hp, prev):
                # scores+exp for pair hp, with the PV matmuls of the previous
                # pair interleaved into the same jc loop so the PE never
                # stalls on ACT's exp backlog. Both heads of the pair share
                # one [128,512] PSUM bank -> one exp ACTIVATE per block.
                e = lp.tile([P, JC, 512], bf16, name=f"e{hp}", tag="expT",
                            bufs=3)
                if prev is not None:
                    hq, eq = prev
                    pvs = [pp.tile([DH + 1, RV], f32, name=f"pv{2 * hq + hh}",
                                   tag="pv", bufs=3) for hh in range(2)]
                for jc in range(JC):
                    for hh in range(2):
                        lo = hh * 64
                        sc = pp.tile([P, RV], f32, name=f"sc{hp}_{jc}_{hh}",
                                     tag="big", bufs=3)
                        nc.tensor.matmul(
                            sc[:],
                            kT[lo:lo + 64, hp, jc * P:(jc + 1) * P],
                            qT[lo:lo + 64, hp, :],
                            start=True, stop=True)
                        nc.scalar.activation(
                            e[:, jc, hh * RV:(hh + 1) * RV], sc[:],
                            AF.Exp, scale=SCALE)
                    if prev is not None:
                        for hh in range(2):
                            nc.tensor.matmul(
                                pvs[hh][:], v_aug[:, jc, 2 * hq + hh, :],
                                eq[:, jc, hh * RV:(hh + 1) * RV],
                                start=(jc == 0), stop=(jc == JC - 1))
                if prev is not None:
                    pair_tail(hq, pvs)
                return e

            def last_pv(hq, eq):
                pvs = [pp.tile([DH + 1, RV], f32, name=f"pv{2 * hq + hh}",
                               tag="pv", bufs=3) for hh in range(2)]
                for jc in range(JC):
                    for hh in range(2):
                        nc.tensor.matmul(
                            pvs[hh][:], v_aug[:, jc, 2 * hq + hh, :],
                            eq[:, jc, hh * RV:(hh + 1) * RV],
                            start=(jc == 0), stop=(jc == JC - 1))
                pair_tail(hq, pvs)

            kproj(0)
            kproj(1)
            kproj(2)
            vproj(0)
            e0 = pair_block(0, None)
            m32 = vchain_cm(ybt, w_dv, b_dv, "m")
            m_cm = ap.tile([P, KC], bf16, name="m_cm")
            nc.vector.tensor_copy(m_cm[:], m32[:])
            e1h = pair_block(1, (0, e0))
            e2h = pair_block(2, (1, e1h))
            w_vfc = wtile("van_fc_w")
            b_vfc = btile("van_fc_b")
            van_partial(0, w_vfc, acc_t2, "t2")
            kproj(3)
            kproj(4)
            kproj(5)
            vproj(1)
            # gating-weight prefetch: issued mid-attention in need-order.
            # With bufs=6 the slot gates (6-back readers) all resolve at or
            # before each weight's emission point, so the in-order sync DMA
            # queue never blocks ahead of the z1 trigger.
            w_dth1 = wtile("d_theta_w", half=1)
            w_WV = wtile("WV_w")
            w_vg0 = wtile("v_gamma_w", half=0)
            w_vo0 = wtile("van_out_w", half=0)
            b_vo = btile("van_out_b")
            b_dfc = btile("diff_fc_b")
            ws_gate = wsp.tile([P, 2 * KC, 1], bf16, name="ws_gate", tag="ws")
            nc.sync.dma_start(ws_gate[:], wd["gate_w"].rearrange(
                "(c p) o -> p c o", p=P))
            ws_nf = wsp.tile([P, 2 * KC, 1], bf16, name="ws_nf", tag="ws")
            nc.sync.dma_start(ws_nf[:], wd["nf_out_w"].rearrange(
                "(c p) o -> p c o", p=P))
            bias2 = vchain_cm(m_cm, w_do0, b_dout, "bias2")
            van_partial(1, w_vfc, acc_t2, "t2")
            w_dfc = wtile("diff_fc_w")
            e3h = pair_block(3, (2, e2h))
            van_partial(2, w_vfc, acc_t2, "t2")
            b_dfus = btile("diff_fus_b")
            b_vfus = btile("van_fus_b")
            b_nf = btile("nf_b")
            b_fin = btile("final_b")
            e4h = pair_block(4, (3, e3h))
            van_partial(3, w_vfc, acc_t2, "t2")
            e5h = pair_block(5, (4, e4h))
            van_partial(4, w_vfc, acc_t2, "t2")
            last_pv(5, e5h)
            van_partial(5, w_vfc, acc_t2, "t2")
            # vg1 slot-gates on van_partial(5) (vfc's last read): emit here so
            # the queue unblocks right away; it lands during the e1 gemm.
            w_vg1 = wtile("v_gamma_w", half=1)

            # ---------------- gating network --------------------------------
            def gemm(pairs, func, bias_t=None, accum_t=None, name="g",
                     out_dt=bf16, pre=None):
                out = ap.tile([P, KC, RV], out_dt, name=name)
                nmm = len(pairs) * KC
                for mc in range(KC):
                    ps = pp.tile([P, RV], f32, name=f"{name}ps{mc}", tag="big",
                                 bufs=3)
                    i = 0
                    for wt, at in pairs:
                        for kc in range(KC):
                            nc.tensor.matmul(ps[:],
                                             wt[:, kc, mc * P:(mc + 1) * P],
                                             at[:, kc, :],
                                             start=(i == 0), stop=(i == nmm - 1))
                            i += 1
                    src = ps
                    if pre is not None:
                        tmp = lp.tile([P, RV], f32, name=f"{name}pre{mc}",
                                      tag="pretmp")
                        nc.vector.tensor_add(tmp[:], ps[:], pre[:, mc, :])
                        src = tmp
                    if func == AF.Identity and accum_t is None:
                        if bias_t is not None:
                            nc.vector.tensor_scalar_add(out[:, mc, :], src[:],
                                                        bias_t[:, mc:mc + 1])
                        else:
                            nc.vector.tensor_copy(out[:, mc, :], src[:])
                    else:
                        nc.scalar.activation(
                            out[:, mc, :], src[:], func,
                            bias=(bias_t[:, mc:mc + 1] if bias_t is not None
                                  else 0.0),
                            accum_out=(accum_t[:, mc:mc + 1]
                                       if accum_t is not None else None))
                return out

            def ag_start(part, name):
                gi = dp.tile([P, KC], f32, name=f"gi_{name}")
                go = dp.tile([4 * P, KC], f32, name=f"go_{name}")
                nc.sync.dma_start(gi[:], part[:])
                nc.gpsimd.collective_compute(
                    "AllGather", ALU.bypass, replica_groups=GROUPS,
                    ins=[gi[:]], outs=[go[:]])
                return go

            def ag_finish(go, name):
                zt = ap.tile([P, 4, KC], f32, name=f"zt_{name}")
                nc.sync.dma_start(zt[:], go.rearrange("(r p) c -> p r c", p=P))
                z = ap.tile([P, KC], f32, name=f"z_{name}")
                nc.vector.tensor_add(z[:], zt[:, 0, :], zt[:, 1, :])
                nc.vector.tensor_add(z[:], z[:], zt[:, 2, :])
                nc.vector.tensor_add(z[:], z[:], zt[:, 3, :])
                return z

            theta2 = ap.tile([P, KC, RV], bf16, name="theta2")
            for mc in range(KC):
                nc.scalar.activation(theta2[:, mc, :], acc_t2[:, mc, :],
                                     AF.Tanh, bias=b_vfc[:, mc:mc + 1])

            part1 = ap.tile([P, KC], f32, name="part1")
            e1 = gemm([(w_dth1, theta2)], AF.Exp, accum_t=part1, name="e1")
            go1 = ag_start(part1, "z1")

            # --- AllGather-1 bubble fillers (independent of z1) -------------
            # gamma1 and voa interleaved at the mc level so each one's ACT
            # epilogues hide under the other's matmuls. Weight DMAs for the
            # post-z1 GEMMs are emitted here in need-order; each slot gate
            # resolves no later than the previous one (monotone), so the
            # queue drains without head-of-line blocking.
            w_do1 = wtile("diff_out_w", half=1)
            gamma1 = ap.tile([P, KC, RV], bf16, name="gamma1")
            voa = ap.tile([P, KC, RV], f32, name="voa")
            for mc in range(KC):
                ps1 = pp.tile([P, RV], f32, name=f"g1ps{mc}", tag="big",
                              bufs=3)
                for kc in range(KC):
                    nc.tensor.matmul(ps1[:], w_WV[:, kc, mc * P:(mc + 1) * P],
                                     vanT[:, kc, :],
                                     start=(kc == 0), stop=(kc == KC - 1))
                nc.scalar.activation(gamma1[:, mc, :], ps1[:], AF.Tanh)
                ps2 = pp.tile([P, RV], f32, name=f"voaps{mc}", tag="big",
                              bufs=3)
                for kc in range(KC):
                    nc.tensor.matmul(ps2[:], w_vo0[:, kc, mc * P:(mc + 1) * P],
                                     vanT[:, kc, :],
                                     start=(kc == 0), stop=(kc == KC - 1))
                nc.vector.tensor_scalar_add(voa[:, mc, :], ps2[:],
                                            b_vo[:, mc:mc + 1])
            w_dfus = wtile("diff_fus_w")
            z2a = gemm([(w_vg0, gamma1)], AF.Identity, name="z2a", out_dt=f32)
            w_vo1 = wtile("van_out_w", half=1)
            ps_nf = pp.tile([1, RV], f32, name="nfps", tag="sps", bufs=2)
            for kc in range(KC):
                nc.tensor.matmul(ps_nf[:], ws_nf[:, kc, :], vanT[:, kc, :],
                                 start=(kc == 0), stop=False,
                                 skip_group_check=True)
            w_vfus = wtile("van_fus_w")
            z1 = ag_finish(go1, "z1")

            s1 = ap.tile([P, KC], f32, name="s1")
            nc.vector.reciprocal(s1[:], z1[:])
            nc.vector.tensor_mul(s1[:], s1[:], m32[:])
            dth = ap.tile([P, KC, RV], bf16, name="dth")
            for mc in range(KC):
                nc.vector.tensor_scalar_mul(dth[:, mc, :], e1[:, mc, :],
                                            s1[:, mc:mc + 1])

            gamma2 = gemm([(w_dfc, dth)], AF.Tanh, bias_t=b_dfc, name="gamma2")
            w_nf = wtile("nf_w")
            part2 = ap.tile([P, KC], f32, name="part2")
            e2 = gemm([(w_vg1, gamma2)], AF.Exp, accum_t=part2, pre=z2a,
                      name="e2")
            go2 = ag_start(part2, "z2")
            w_fin = wtile("final_w")

            # --- AllGather-2 bubble fillers --------------------------------
            dout = gemm([(w_do1, dth)], AF.Tanh, bias_t=bias2, name="dout")
            dfus = gemm([(w_dfus, dout)], AF.Tanh, bias_t=b_dfus, name="dfus")
            z2 = ag_finish(go2, "z2")

            s2 = ap.tile([P, KC], f32, name="s2")
            nc.vector.reciprocal(s2[:], z2[:])
            ag = ap.tile([P, KC, RV], bf16, name="ag")
            for mc in range(KC):
                nc.vector.scalar_tensor_tensor(
                    ag[:, mc, :], e2[:, mc, :], s2[:, mc:mc + 1],
                    vanT[:, mc, :], op0=ALU.mult, op1=ALU.mult)

            vout = gemm([(w_vo1, ag)], AF.Tanh, pre=voa, name="vout")
            vfus = gemm([(w_vfus, vout)], AF.Tanh, bias_t=b_vfus, name="vfus")
            diffv = ap.tile([P, KC, RV], bf16, name="diffv")
            for mc in range(KC):
                nc.vector.tensor_sub(diffv[:, mc, :], vfus[:, mc, :],
                                     dfus[:, mc, :])

            # gate (M=1 GEMM over both fusion tensors)
            ps_g = pp.tile([1, RV], f32, name="gateps", tag="sps", bufs=2)
            i = 0
            for at, base in [(dfus, 0), (vfus, KC)]:
                for kc in range(KC):
                    nc.tensor.matmul(ps_g[:], ws_gate[:, base + kc, :],
                                     at[:, kc, :],
                                     start=(i == 0), stop=(i == 2 * KC - 1))
                    i += 1
            gb16 = ap.tile([1, RV], bf16, name="gb16")
            nc.scalar.activation(gb16[:], ps_g[:], AF.Sigmoid)
            gbc = pp.tile([P, RV], f32, name="gbc", tag="pv", bufs=3)
            nc.tensor.matmul(gbc[:], ones128[:], gb16[:], start=True, stop=True)

            fus = ap.tile([P, KC, RV], bf16, name="fus")
            for mc in range(KC):
                t2 = lp.tile([P, RV], bf16, name=f"ft2_{mc}", tag="ft2")
                nc.vector.tensor_mul(t2[:], diffv[:, mc, :], gbc[:])
                nc.vector.tensor_add(fus[:, mc, :], t2[:], dfus[:, mc, :])

            # tnf first, so the nf sigmoid/broadcast is ready before the
            # final tanh GEMM and its fused mul+store epilogue.
            tnf = gemm([(w_nf, fus)], AF.Identity, bias_t=b_nf, name="tnf")
            for kc in range(KC):
                nc.tensor.matmul(ps_nf[:], ws_nf[:, KC + kc, :], tnf[:, kc, :],
                                 start=False, stop=(kc == KC - 1),
                                 skip_group_check=True)
            nb16 = ap.tile([1, RV], bf16, name="nb16")
            nc.scalar.activation(nb16[:], ps_nf[:], AF.Sigmoid)
            nbc = pp.tile([P, RV], f32, name="nbc", tag="pv", bufs=3)
            nc.tensor.matmul(nbc[:], ones128[:], nb16[:], start=True, stop=True)

            od = out_d.rearrange("(mc p) n -> mc p n", p=P)
            for mc in range(KC):
                ps = pp.tile([P, RV], f32, name=f"ftps{mc}", tag="big", bufs=3)
                for kc in range(KC):
                    nc.tensor.matmul(ps[:], w_fin[:, kc, mc * P:(mc + 1) * P],
                                     fus[:, kc, :],
                                     start=(kc == 0), stop=(kc == KC - 1))
                ftc = lp.tile([P, RV], bf16, name=f"ftc{mc}", tag="ftc")
                nc.scalar.activation(ftc[:], ps[:], AF.Tanh,
                                     bias=b_fin[:, mc:mc + 1])
                ot = lp.tile([P, RV], f32, name=f"ot{mc}", tag="ot", bufs=3)
                nc.vector.tensor_mul(ot[:], ftc[:], nbc[:])
                nc.sync.dma_start(od[mc], ot[:])

    nc.compile()
    return nc


_CACHE = {}


def _prep_in_maps(inputs):
    x = np.asarray(inputs["x"], np.float32)
    y = np.asarray(inputs["y"], np.float32)
    has_vvb = bool(np.any(np.asarray(inputs["vv_b"]) != 0))

    xt = np.ascontiguousarray(x.reshape(B * S, H).T).astype(nbf16)   # [H, 2048]
    yts = [np.ascontiguousarray(y[b].T).astype(nbf16) for b in range(B)]

    base = {}
    for w in W768 + W1536 + ["gate_w", "nf_out_w"]:
        base[w] = np.asarray(inputs[w], np.float32).astype(nbf16)
    for b in BIAS:
        base[b] = np.ascontiguousarray(np.asarray(inputs[b], np.float32))
    if has_vvb:
        base["vv_b"] = np.ascontiguousarray(np.asarray(inputs["vv_b"], np.float32))

    in_maps = []
    for c in range(8):
        bat = c // 4
        m = dict(base)
        m["xT"] = np.ascontiguousarray(xt[:, c * RV:(c + 1) * RV])
        m["yT"] = yts[bat]
        in_maps.append(m)
    return in_maps, has_vvb


def kernel(**inputs):
    in_maps, has_vvb = _prep_in_maps(inputs)
    if has_vvb not in _CACHE:
        _CACHE[has_vvb] = build(has_vvb)
    nc = _CACHE[has_vvb]

    res = run_bass_kernel_spmd(nc, in_maps, core_ids=list(range(8)))
    full = np.concatenate([res.results[c]["outT"] for c in range(8)], axis=1)
    return np.ascontiguousarray(full.T.reshape(B, S, H)).astype(np.float32)


if __name__ == "__main__":
    rng = np.random.default_rng(0)
    ins = {"x": rng.standard_normal((B, S, H)).astype(np.float32),
           "y": rng.standard_normal((B, S, H)).astype(np.float32)}
    for w in W768 + W1536 + ["dq_w", "dk_w", "WD_w"]:
        shp = (H, H) if w not in W1536 else (2 * H, H)
        ins[w] = (rng.standard_normal(shp) * 0.02).astype(np.float32)
    ins["gate_w"] = (rng.standard_normal((2 * H, 1)) * 0.02).astype(np.float32)
    ins["nf_out_w"] = (rng.standard_normal((2 * H, 1)) * 0.02).astype(np.float32)
    for b in BIAS + ["vv_b", "dq_b", "dk_b", "d_theta_b", "v_gamma_b"]:
        ins[b] = np.zeros(H, np.float32)
    out = kernel(**ins)
    print("out", out.shape, out.dtype, np.abs(out).mean())


# revision 40
# speedup vs baseline: 1.1278x; 1.0052x over previous
"""Coupled-attention module as a distributed Bass/Tile kernel on 8 TRN2 cores.

Math notes (exact algebra, not approximations):
- The differential-attention scores are constant along the softmax axis, so
  softmax yields exactly uniform 1/S weights: diff_vector collapses to the
  per-batch mean of (y @ dv_w + dv_b), broadcast over sequence. dq/dk are dead.
- The two gating softmaxes run over the sequence axis (dim=1). Terms constant
  along that axis cancel in softmax exactly: d_theta_b and th1 @ d_theta_w[:H]
  (diff branch) and v_gamma_b (van branch) are all dead. This kills the whole
  th1/bias1 chain and the WD_w / d_theta_w[:H] weights.
- Sharding: rows of the flattened (B*S, H) activations, 256 per core; cores
  0-3 own batch 0, 4-7 batch 1. Each core redundantly computes full-batch K/V
  (collective reshards measure slower than the redundant GEMMs on this part).
- Attention head pairs are packed into disjoint PE row groups (K=64 each); the
  two scores matmuls of a pair write disjoint halves of one [128,512] PSUM
  bank and a single exp ACTIVATE covers both heads.
- The two sequence-axis softmax denominators are summed across the 4-core
  batch group with small AllGathers + local adds. The gathers are split into
  trigger (right after the partial sums) and finish (after the filler GEMMs),
  and every DMA needed before a gather resolves is enqueued ahead of it: the
  sync DMA queue is strictly in-order, so a descriptor gated on the collective
  would head-of-line block all later weight loads (measured 26us PE stall).
- Weight loads use one dma_start per tile (the sync queue costs ~600ns per
  entry) and are prefetched in need-order; the order is chosen so weight-pool
  slot recycling (bufs=5) never gates a DMA on a reader that runs later.
- Compute in bf16 with fp32 accumulation; exp/tanh/sigmoid on ACT; identity
  epilogues on DVE to keep ACT for transcendentals.
"""

import numpy as np
import ml_dtypes

import concourse.bass as bass
import concourse.mybir as mybir
import concourse.tile as tile
from concourse import bacc
from concourse.bass_utils import run_bass_kernel_spmd

B, S, H = 2, 1024, 768
NH, DH = 12, 64
P = 128
RV = 256            # rows per core
KC = H // P         # 6 channel chunks
JC = S // P         # 8 sequence chunks
GROUPS = [[0, 1, 2, 3], [4, 5, 6, 7]]
SCALE = 1.0 / 8.0   # 1/sqrt(DH)

bf16 = mybir.dt.bfloat16
f32 = mybir.dt.float32
AF = mybir.ActivationFunctionType
ALU = mybir.AluOpType
nbf16 = ml_dtypes.bfloat16

W768 = ["vq_w", "vk_w", "vv_w", "dv_w", "van_fc_w", "WV_w", "diff_fc_w",
        "diff_fus_w", "van_fus_w", "nf_w", "final_w"]
W1536 = ["d_theta_w", "v_gamma_w", "diff_out_w", "van_out_w"]
BIAS = ["vq_b", "vk_b", "dv_b", "van_fc_b", "diff_fc_b",
        "diff_out_b", "van_out_b", "diff_fus_b", "van_fus_b",
        "nf_b", "final_b"]


def build(has_vvb: bool):
    nc = bacc.Bacc(None, target_bir_lowering=False, debug=False, num_devices=8)

    xT_d = nc.dram_tensor("xT", [H, RV], bf16, kind="ExternalInput")
    yT_d = nc.dram_tensor("yT", [H, S], bf16, kind="ExternalInput")
    wd = {}
    for w in W768:
        wd[w] = nc.dram_tensor(w, [H, H], bf16, kind="ExternalInput")
    for w in W1536:
        wd[w] = nc.dram_tensor(w, [2 * H, H], bf16, kind="ExternalInput")
    wd["gate_w"] = nc.dram_tensor("gate_w", [2 * H, 1], bf16, kind="ExternalInput")
    wd["nf_out_w"] = nc.dram_tensor("nf_out_w", [2 * H, 1], bf16, kind="ExternalInput")
    bd = {}
    for b in BIAS:
        bd[b] = nc.dram_tensor(b, [H], f32, kind="ExternalInput")
    if has_vvb:
        bd["vv_b"] = nc.dram_tensor("vv_b", [H], f32, kind="ExternalInput")
    out_d = nc.dram_tensor("outT", [H, RV], f32, kind="ExternalOutput")

    with tile.TileContext(nc, num_cores=8) as tc:
        with (
            tc.tile_pool(name="wpool", bufs=6) as wp,
            tc.tile_pool(name="wsmall", bufs=2) as wsp,
            tc.tile_pool(name="acts", bufs=1) as ap,
            tc.tile_pool(name="loop", bufs=2) as lp,
            tc.tile_pool(name="psum", bufs=8, space="PSUM") as pp,
            tc.tile_pool(name="dram", bufs=1, space="DRAM") as dp,
        ):
            def wtile(name, half=None, split=False):
                t = wp.tile([P, KC, H], bf16, name=f"w_{name}_{half}", tag="w")
                src = wd[name]
                if half is not None:
                    src = src[half * H:(half + 1) * H, :]
                src = src.rearrange("(kc p) n -> kc p n", p=P)
                for kc in range(KC):
                    nc.sync.dma_start(t[:, kc, :], src[kc])
                return t

            def btile(name):
                t = ap.tile([P, KC], f32, name=f"b_{name}")
                nc.sync.dma_start(t[:], bd[name].rearrange("(c p) -> p c", p=P))
                return t

            def brow(name):
                t = ap.tile([1, H], f32, name=f"br_{name}")
                nc.sync.dma_start(t[:], bd[name].rearrange("(o c) -> o c", o=1))
                return t

            # ---------------- Q projection first: minimal-dependency PE work
            b_vq = btile("vq_b")
            xT = ap.tile([P, KC, RV], bf16, name="xT")
            for kc in range(KC):
                nc.sync.dma_start(xT[:, kc, :], xT_d.rearrange(
                    "(kc p) n -> kc p n", p=P)[kc])
            w_vq = wtile("vq_w")
            b_vk = btile("vk_b")
            w_vk = wtile("vk_w")
            yT = ap.tile([P, KC, S], bf16, name="yT")
            for kc in range(KC):
                nc.sync.dma_start(yT[:, kc, :], yT_d.rearrange(
                    "(kc p) n -> kc p n", p=P)[kc])
            w_vv = wtile("vv_w")

            # warm up the collective stream early: the first real collective
            # otherwise pays an ~24us trigger-start delay.
            dgi = dp.tile([P, 1], f32, name="dgi")
            dgo = dp.tile([4 * P, 1], f32, name="dgo")
            nc.sync.dma_start(dgi[:], b_vq[:, 0:1])
            nc.gpsimd.collective_compute(
                "AllGather", ALU.bypass, replica_groups=GROUPS,
                ins=[dgi[:]], outs=[dgo[:]])

            qT = ap.tile([P, KC, RV], bf16, name="qT")
            for mc in range(KC):
                ps = pp.tile([P, RV], f32, name=f"qps{mc}", tag="big", bufs=3)
                for kc in range(KC):
                    nc.tensor.matmul(ps[:], w_vq[:, kc, mc * P:(mc + 1) * P],
                                     xT[:, kc, :],
                                     start=(kc == 0), stop=(kc == KC - 1))
                nc.vector.tensor_scalar_add(qT[:, mc, :], ps[:],
                                            b_vq[:, mc:mc + 1])

            ones64 = ap.tile([1, 64], bf16, name="ones64")
            nc.vector.memset(ones64[:], 1.0)
            ones128 = ap.tile([1, P], bf16, name="ones128")
            nc.vector.memset(ones128[:], 1.0)

            # ---------------- K/V projections, emitted in two halves so the
            # attention pairs of the first half overlap the second half ------
            kT = ap.tile([P, KC, S], bf16, name="kT")
            v_aug = ap.tile([P, JC, NH, DH + 1], bf16, name="v_aug")
            nc.vector.memset(v_aug[:, :, :, DH:DH + 1], 1.0)

            # Epilogues of k/v projections run on GPSIMD (otherwise idle):
            # the scores/PV matmuls gate on these, and DVE was measured as
            # the attention-phase serializer (17.8us of PE stalls).
            def kproj(mc):
                for sh in range(2):
                    ps = pp.tile([P, 512], f32, name=f"kps{mc}_{sh}",
                                 tag="big", bufs=3)
                    for kc in range(KC):
                        nc.tensor.matmul(
                            ps[:], w_vk[:, kc, mc * P:(mc + 1) * P],
                            yT[:, kc, sh * 512:(sh + 1) * 512],
                            start=(kc == 0), stop=(kc == KC - 1))
                    nc.scalar.activation(
                        kT[:, mc, sh * 512:(sh + 1) * 512], ps[:],
                        AF.Identity, bias=b_vk[:, mc:mc + 1])

            def vproj(cg):
                for jc in range(JC):
                    ps = pp.tile([P, 384], f32, name=f"vps{jc}_{cg}",
                                 tag="big", bufs=3)
                    for kc in range(KC):
                        nc.tensor.matmul(
                            ps[:], yT[:, kc, jc * P:(jc + 1) * P],
                            w_vv[:, kc, cg * 384:(cg + 1) * 384],
                            start=(kc == 0), stop=(kc == KC - 1))
                    nc.vector.tensor_copy(
                        v_aug[:, jc, cg * 6:(cg + 1) * 6, 0:DH],
                        ps[:].rearrange("p (h d) -> p h d", d=DH))

            # ---------------- per-batch chain pieces ------------------------
            yb = ap.tile([P, KC], f32, name="yb")
            for kc in range(KC):
                nc.vector.tensor_reduce(yb[:, kc:kc + 1], yT[:, kc, :],
                                        axis=mybir.AxisListType.X, op=ALU.add)
            ybt = ap.tile([P, KC], bf16, name="ybt")
            nc.vector.tensor_scalar_mul(ybt[:], yb[:], 1.0 / S)

            def vchain_cm(vec_cm, w_t, bias_cm, name):
                # chan-major out [128, 6] f32 = vec @ W + bias, computed
                # directly in chan-major form (stationary = weight chunk,
                # moving = the [128,1] chan-major vector chunk): no DRAM
                # bounce, so the sync DMA queue never blocks on PE progress.
                out = ap.tile([P, KC], f32, name=f"{name}_cm32")
                for mc in range(KC):
                    ps = pp.tile([P, 1], f32, name=f"{name}ps{mc}",
                                 tag="sps", bufs=2)
                    for kc in range(KC):
                        nc.tensor.matmul(ps[:],
                                         w_t[:, kc, mc * P:(mc + 1) * P],
                                         vec_cm[:, kc:kc + 1],
                                         start=(kc == 0), stop=(kc == KC - 1))
                    if bias_cm is not None:
                        nc.vector.tensor_add(out[:, mc:mc + 1], ps[:],
                                             bias_cm[:, mc:mc + 1])
                    else:
                        nc.vector.tensor_copy(out[:, mc:mc + 1], ps[:])
                return out

            # ---------------- attention: pairs pipelined against K/V -------
            w_dv = wtile("dv_w")
            b_dv = btile("dv_b")
            w_do0 = wtile("diff_out_w", half=0)
            b_dout = btile("diff_out_b")
            if has_vvb:
                b_vv = btile("vv_b")
            vanT = ap.tile([P, KC, RV], bf16, name="vanT")
            acc_t2 = ap.tile([P, KC, RV], f32, name="acc_t2")

            def van_partial(kc, w_t, acc, tg):
                # acc += w[kc-chunk].T @ vanT[kc] as soon as the pair-kc
                # attention output lands; accumulate in SBUF f32 on GPSIMD.
                for mc in range(KC):
                    ps = pp.tile([P, RV], f32, name=f"{tg}p{kc}_{mc}",
                                 tag="pv", bufs=3)
                    nc.tensor.matmul(ps[:], w_t[:, kc, mc * P:(mc + 1) * P],
                                     vanT[:, kc, :], start=True, stop=True)
                    if kc == 0:
                        nc.vector.tensor_copy(acc[:, mc, :], ps[:])
                    else:
                        nc.vector.tensor_add(acc[:, mc, :], acc[:, mc, :],
                                             ps[:])

            def pair_tail(hq, pvs):
                hp = hq
                for hh in range(2):
                    invZb = lp.tile([1, RV], bf16, name=f"invZb{2 * hq + hh}",
                                    tag="invZb")
                    with nc.allow_low_precision(reason="1/Z feeds bf16 mul"):
                        nc.vector.reciprocal(invZb[:], pvs[hh][DH:DH + 1, :])
                    bc = pp.tile([64, RV], f32, name=f"bc{2 * hq + hh}",
                                 tag="big", bufs=3)
                    nc.tensor.matmul(bc[:], ones64[:], invZb[:],
                                     start=True, stop=True)
                    bcs = lp.tile([64, RV], bf16, name=f"bcs{2 * hq + hh}",
                                  tag="bcs")
                    nc.vector.tensor_copy(bcs[:], bc[:])
                    nc.vector.tensor_mul(vanT[hh * 64:hh * 64 + 64, hp, :],
                                         pvs[hh][0:DH, :], bcs[:])
                    if has_vvb:
                        nc.vector.tensor_scalar_add(
                            vanT[hh * 64:hh * 64 + 64, hp, :],
                            vanT[hh * 64:hh * 64 + 64, hp, :],
                            b_vv[hh * 64:hh * 64 + 64, hp:hp + 1])

            def pair_block(hp, prev):
                # scores+exp for pair hp, with the PV matmuls of the previous
                # pair interleaved into the same jc loop so the PE never
                # stalls on ACT's exp backlog. Both heads of the pair share
                # one [128,512] PSUM bank -> one exp ACTIVATE per block.
                e = lp.tile([P, JC, 512], bf16, name=f"e{hp}", tag="expT",
                            bufs=3)
                if prev is not None:
                    hq, eq = prev
                    pvs = [pp.tile([DH + 1, RV], f32, name=f"pv{2 * hq + hh}",
                                   tag="pv", bufs=3) for hh in range(2)]
                for jc in range(JC):
                    for hh in range(2):
                        lo = hh * 64
                        sc = pp.tile([P, RV], f32, name=f"sc{hp}_{jc}_{hh}",
                                     tag="big", bufs=3)
                        nc.tensor.matmul(
                            sc[:],
                            kT[lo:lo + 64, hp, jc * P:(jc + 1) * P],
                            qT[lo:lo + 64, hp, :],
                            start=True, stop=True)
                        nc.scalar.activation(
                            e[:, jc, hh * RV:(hh + 1) * RV], sc[:],
                            AF.Exp, scale=SCALE)
                    if prev is not None:
                        for hh in range(2):
                            nc.tensor.matmul(
                                pvs[hh][:], v_aug[:, jc, 2 * hq + hh, :],
                                eq[:, jc, hh * RV:(hh + 1) * RV],
                                start=(jc == 0), stop=(jc == JC - 1))
                if prev is not None:
                    pair_tail(hq, pvs)
                return e

            def last_pv(hq, eq):
                pvs = [pp.tile([DH + 1, RV], f32, name=f"pv{2 * hq + hh}",
                               tag="pv", bufs=3) for hh in range(2)]
                for jc in range(JC):
                    for hh in range(2):
                        nc.tensor.matmul(
                            pvs[hh][:], v_aug[:, jc, 2 * hq + hh, :],
                            eq[:, jc, hh * RV:(hh + 1) * RV],
                            start=(jc == 0), stop=(jc == JC - 1))
                pair_tail(hq, pvs)

            kproj(0)
            kproj(1)
            kproj(2)
            vproj(0)
            e0 = pair_block(0, None)
            m32 = vchain_cm(ybt, w_dv, b_dv, "m")
            m_cm = ap.tile([P, KC], bf16, name="m_cm")
            nc.vector.tensor_copy(m_cm[:], m32[:])
            e1h = pair_block(1, (0, e0))
            e2h = pair_block(2, (1, e1h))
            w_vfc = wtile("van_fc_w")
            b_vfc = btile("van_fc_b")
            van_partial(0, w_vfc, acc_t2, "t2")
            kproj(3)
            kproj(4)
            kproj(5)
            vproj(1)
            # gating-weight prefetch: issued mid-attention in need-order.
            # With bufs=6 the slot gates (6-back readers) all resolve at or
            # before each weight's emission point, so the in-order sync DMA
            # queue never blocks ahead of the z1 trigger.
            w_dth1 = wtile("d_theta_w", half=1)
            w_WV = wtile("WV_w")
            w_vg0 = wtile("v_gamma_w", half=0)
            w_vo0 = wtile("van_out_w", half=0)
            b_vo = btile("van_out_b")
            b_dfc = btile("diff_fc_b")
            ws_gate = wsp.tile([P, 2 * KC, 1], bf16, name="ws_gate", tag="ws")
            nc.sync.dma_start(ws_gate[:], wd["gate_w"].rearrange(
                "(c p) o -> p c o", p=P))
            ws_nf = wsp.tile([P, 2 * KC, 1], bf16, name="ws_nf", tag="ws")
            nc.sync.dma_start(ws_nf[:], wd["nf_out_w"].rearrange(
                "(c p) o -> p c o", p=P))
            bias2 = vchain_cm(m_cm, w_do0, b_dout, "bias2")
            van_partial(1, w_vfc, acc_t2, "t2")
            w_dfc = wtile("diff_fc_w")
            e3h = pair_block(3, (2, e2h))
            van_partial(2, w_vfc, acc_t2, "t2")
            b_dfus = btile("diff_fus_b")
            b_vfus = btile("van_fus_b")
            b_nf = btile("nf_b")
            b_fin = btile("final_b")
            e4h = pair_block(4, (3, e3h))
            van_partial(3, w_vfc, acc_t2, "t2")
            e5h = pair_block(5, (4, e4h))
            van_partial(4, w_vfc, acc_t2, "t2")
            last_pv(5, e5h)
            van_partial(5, w_vfc, acc_t2, "t2")
            # vg1 slot-gates on van_partial(5) (vfc's last read): emit here so
            # the queue unblocks right away; it lands during the e1 gemm.
            w_vg1 = wtile("v_gamma_w", half=1)

            # ---------------- gating network --------------------------------
            def gemm(pairs, func, bias_t=None, accum_t=None, name="g",
                     out_dt=bf16, pre=None):
                out = ap.tile([P, KC, RV], out_dt, name=name)
                nmm = len(pairs) * KC
                for mc in range(KC):
                    ps = pp.tile([P, RV], f32, name=f"{name}ps{mc}", tag="big",
                                 bufs=3)
                    i = 0
                    for wt, at in pairs:
                        for kc in range(KC):
                            nc.tensor.matmul(ps[:],
                                             wt[:, kc, mc * P:(mc + 1) * P],
                                             at[:, kc, :],
                                             start=(i == 0), stop=(i == nmm - 1))
                            i += 1
                    src = ps
                    if pre is not None:
                        tmp = lp.tile([P, RV], f32, name=f"{name}pre{mc}",
                                      tag="pretmp")
                        nc.vector.tensor_add(tmp[:], ps[:], pre[:, mc, :])
                        src = tmp
                    if func == AF.Identity and accum_t is None:
                        if bias_t is not None:
                            nc.vector.tensor_scalar_add(out[:, mc, :], src[:],
                                                        bias_t[:, mc:mc + 1])
                        else:
                            nc.vector.tensor_copy(out[:, mc, :], src[:])
                    else:
                        nc.scalar.activation(
                            out[:, mc, :], src[:], func,
                            bias=(bias_t[:, mc:mc + 1] if bias_t is not None
                                  else 0.0),
                            accum_out=(accum_t[:, mc:mc + 1]
                                       if accum_t is not None else None))
                return out

            def ag_start(part, name):
                gi = dp.tile([P, KC], f32, name=f"gi_{name}")
                go = dp.tile([4 * P, KC], f32, name=f"go_{name}")
                nc.sync.dma_start(gi[:], part[:])
                nc.gpsimd.collective_compute(
                    "AllGather", ALU.bypass, replica_groups=GROUPS,
                    ins=[gi[:]], outs=[go[:]])
                return go

            def ag_finish(go, name):
                zt = ap.tile([P, 4, KC], f32, name=f"zt_{name}")
                nc.sync.dma_start(zt[:], go.rearrange("(r p) c -> p r c", p=P))
                z = ap.tile([P, KC], f32, name=f"z_{name}")
                nc.vector.tensor_add(z[:], zt[:, 0, :], zt[:, 1, :])
                nc.vector.tensor_add(z[:], z[:], zt[:, 2, :])
                nc.vector.tensor_add(z[:], z[:], zt[:, 3, :])
                return z

            theta2 = ap.tile([P, KC, RV], bf16, name="theta2")
            for mc in range(KC):
                nc.scalar.activation(theta2[:, mc, :], acc_t2[:, mc, :],
                                     AF.Tanh, bias=b_vfc[:, mc:mc + 1])

            part1 = ap.tile([P, KC], f32, name="part1")
            e1 = gemm([(w_dth1, theta2)], AF.Exp, accum_t=part1, name="e1")
            go1 = ag_start(part1, "z1")
            # readback enqueued right behind the trigger: anything between
            # them would delay the gather result by ~600ns per queue entry.
            z1 = ag_finish(go1, "z1")

            # --- AllGather-1 bubble fillers (independent of z1) -------------
            # gamma1 and voa interleaved at the mc level so each one's ACT
            # epilogues hide under the other's matmuls. Weight DMAs for the
            # post-z1 GEMMs are emitted here in need-order; each slot gate
            # resolves no later than the previous one (monotone), so the
            # queue drains without head-of-line blocking.
            w_do1 = wtile("diff_out_w", half=1)
            gamma1 = ap.tile([P, KC, RV], bf16, name="gamma1")
            voa = ap.tile([P, KC, RV], f32, name="voa")
            for mc in range(KC):
                ps1 = pp.tile([P, RV], f32, name=f"g1ps{mc}", tag="big",
                              bufs=3)
                for kc in range(KC):
                    nc.tensor.matmul(ps1[:], w_WV[:, kc, mc * P:(mc + 1) * P],
                                     vanT[:, kc, :],
                                     start=(kc == 0), stop=(kc == KC - 1))
                nc.scalar.activation(gamma1[:, mc, :], ps1[:], AF.Tanh)
                ps2 = pp.tile([P, RV], f32, name=f"voaps{mc}", tag="big",
                              bufs=3)
                for kc in range(KC):
                    nc.tensor.matmul(ps2[:], w_vo0[:, kc, mc * P:(mc + 1) * P],
                                     vanT[:, kc, :],
                                     start=(kc == 0), stop=(kc == KC - 1))
                nc.vector.tensor_scalar_add(voa[:, mc, :], ps2[:],
                                            b_vo[:, mc:mc + 1])
            w_dfus = wtile("diff_fus_w")
            z2a = gemm([(w_vg0, gamma1)], AF.Identity, name="z2a", out_dt=f32)
            w_vo1 = wtile("van_out_w", half=1)
            ps_nf = pp.tile([1, RV], f32, name="nfps", tag="sps", bufs=2)
            for kc in range(KC):
                nc.tensor.matmul(ps_nf[:], ws_nf[:, kc, :], vanT[:, kc, :],
                                 start=(kc == 0), stop=False,
                                 skip_group_check=True)
            w_vfus = wtile("van_fus_w")

            s1 = ap.tile([P, KC], f32, name="s1")
            nc.vector.reciprocal(s1[:], z1[:])
            nc.vector.tensor_mul(s1[:], s1[:], m32[:])
            dth = ap.tile([P, KC, RV], bf16, name="dth")
            for mc in range(KC):
                nc.vector.tensor_scalar_mul(dth[:, mc, :], e1[:, mc, :],
                                            s1[:, mc:mc + 1])

            gamma2 = gemm([(w_dfc, dth)], AF.Tanh, bias_t=b_dfc, name="gamma2")
            w_nf = wtile("nf_w")
            part2 = ap.tile([P, KC], f32, name="part2")
            e2 = gemm([(w_vg1, gamma2)], AF.Exp, accum_t=part2, pre=z2a,
                      name="e2")
            go2 = ag_start(part2, "z2")
            z2 = ag_finish(go2, "z2")
            w_fin = wtile("final_w")

            # --- AllGather-2 bubble fillers --------------------------------
            dout = gemm([(w_do1, dth)], AF.Tanh, bias_t=bias2, name="dout")
            dfus = gemm([(w_dfus, dout)], AF.Tanh, bias_t=b_dfus, name="dfus")

            s2 = ap.tile([P, KC], f32, name="s2")
            nc.vector.reciprocal(s2[:], z2[:])
            ag = ap.tile([P, KC, RV], bf16, name="ag")
            for mc in range(KC):
                nc.vector.scalar_tensor_tensor(
                    ag[:, mc, :], e2[:, mc, :], s2[:, mc:mc + 1],
                    vanT[:, mc, :], op0=ALU.mult, op1=ALU.mult)

            vout = gemm([(w_vo1, ag)], AF.Tanh, pre=voa, name="vout")
            vfus = gemm([(w_vfus, vout)], AF.Tanh, bias_t=b_vfus, name="vfus")
            diffv = ap.tile([P, KC, RV], bf16, name="diffv")
            for mc in range(KC):
                nc.vector.tensor_sub(diffv[:, mc, :], vfus[:, mc, :],
                                     dfus[:, mc, :])

            # gate (M=1 GEMM over both fusion tensors)
            ps_g = pp.tile([1, RV], f32, name="gateps", tag="sps", bufs=2)
            i = 0
            for at, base in [(dfus, 0), (vfus, KC)]:
                for kc in range(KC):
                    nc.tensor.matmul(ps_g[:], ws_gate[:, base + kc, :],
                                     at[:, kc, :],
                                     start=(i == 0), stop=(i == 2 * KC - 1))
                    i += 1
            gb16 = ap.tile([1, RV], bf16, name="gb16")
            nc.scalar.activation(gb16[:], ps_g[:], AF.Sigmoid)
            gbc = pp.tile([P, RV], f32, name="gbc", tag="pv", bufs=3)
            nc.tensor.matmul(gbc[:], ones128[:], gb16[:], start=True, stop=True)

            fus = ap.tile([P, KC, RV], bf16, name="fus")
            for mc in range(KC):
                t2 = lp.tile([P, RV], bf16, name=f"ft2_{mc}", tag="ft2")
                nc.vector.tensor_mul(t2[:], diffv[:, mc, :], gbc[:])
                nc.vector.tensor_add(fus[:, mc, :], t2[:], dfus[:, mc, :])

            # tnf first, so the nf sigmoid/broadcast is ready before the
            # final tanh GEMM and its fused mul+store epilogue.
            tnf = gemm([(w_nf, fus)], AF.Identity, bias_t=b_nf, name="tnf")
            for kc in range(KC):
                nc.tensor.matmul(ps_nf[:], ws_nf[:, KC + kc, :], tnf[:, kc, :],
                                 start=False, stop=(kc == KC - 1),
                                 skip_group_check=True)
            nb16 = ap.tile([1, RV], bf16, name="nb16")
            nc.scalar.activation(nb16[:], ps_nf[:], AF.Sigmoid)
            nbc = pp.tile([P, RV], f32, name="nbc", tag="pv", bufs=3)
            nc.tensor.matmul(nbc[:], ones128[:], nb16[:], start=True, stop=True)

            od = out_d.rearrange("(mc p) n -> mc p n", p=P)
            for mc in range(KC):
                ps = pp.tile([P, RV], f32, name=f"ftps{mc}", tag="big", bufs=3)
                for kc in range(KC):
                    nc.tensor.matmul(ps[:], w_fin[:, kc, mc * P:(mc + 1) * P],
                                     fus[:, kc, :],
                                     start=(kc == 0), stop=(kc == KC - 1))
                ftc = lp.tile([P, RV], bf16, name=f"ftc{mc}", tag="ftc")
                nc.scalar.activation(ftc[:], ps[:], AF.Tanh,
                                     bias=b_fin[:, mc:mc + 1])
                ot = lp.tile([P, RV], f32, name=f"ot{mc}", tag="ot", bufs=3)
                nc.vector.tensor_mul(ot[:], ftc[:], nbc[:])
                nc.sync.dma_start(od[mc], ot[:])

    nc.compile()
    return nc


_CACHE = {}


def _prep_in_maps(inputs):
    x = np.asarray(inputs["x"], np.float32)
    y = np.asarray(inputs["y"], np.float32)
    has_vvb = bool(np.any(np.asarray(inputs["vv_b"]) != 0))

    xt = np.ascontiguousarray(x.reshape(B * S, H).T).astype(nbf16)   # [H, 2048]
    yts = [np.ascontiguousarray(y[b].T).astype(nbf16) for b in range(B)]

    base = {}
    for w in W768 + W1536 + ["gate_w", "nf_out_w"]:
        base[w] = np.asarray(inputs[w], np.float32).astype(nbf16)
    for b in BIAS:
        base[b] = np.ascontiguousarray(np.asarray(inputs[b], np.float32))
    if has_vvb:
        base["vv_b"] = np.ascontiguousarray(np.asarray(inputs["vv_b"], np.float32))

    in_maps = []
    for c in range(8):
        bat = c // 4
        m = dict(base)
        m["xT"] = np.ascontiguousarray(xt[:, c * RV:(c + 1) * RV])
        m["yT"] = yts[bat]
        in_maps.append(m)
    return in_maps, has_vvb


def kernel(**inputs):
    in_maps, has_vvb = _prep_in_maps(inputs)
    if has_vvb not in _CACHE:
        _CACHE[has_vvb] = build(has_vvb)
    nc = _CACHE[has_vvb]

    res = run_bass_kernel_spmd(nc, in_maps, core_ids=list(range(8)))
    full = np.concatenate([res.results[c]["outT"] for c in range(8)], axis=1)
    return np.ascontiguousarray(full.T.reshape(B, S, H)).astype(np.float32)


if __name__ == "__main__":
    rng = np.random.default_rng(0)
    ins = {"x": rng.standard_normal((B, S, H)).astype(np.float32),
           "y": rng.standard_normal((B, S, H)).astype(np.float32)}
    for w in W768 + W1536 + ["dq_w", "dk_w", "WD_w"]:
        shp = (H, H) if w not in W1536 else (2 * H, H)
        ins[w] = (rng.standard_normal(shp) * 0.02).astype(np.float32)
    ins["gate_w"] = (rng.standard_normal((2 * H, 1)) * 0.02).astype(np.float32)
    ins["nf_out_w"] = (rng.standard_normal((2 * H, 1)) * 0.02).astype(np.float32)
    for b in BIAS + ["vv_b", "dq_b", "dk_b", "d_theta_b", "v_gamma_b"]:
        ins[b] = np.zeros(H, np.float32)
    out = kernel(**ins)
    print("out", out.shape, out.dtype, np.abs(out).mean())


# revision 42
# speedup vs baseline: 1.1409x; 1.0116x over previous
"""Coupled-attention module as a distributed Bass/Tile kernel on 8 TRN2 cores.

Math notes (exact algebra, not approximations):
- The differential-attention scores are constant along the softmax axis, so
  softmax yields exactly uniform 1/S weights: diff_vector collapses to the
  per-batch mean of (y @ dv_w + dv_b), broadcast over sequence. dq/dk are dead.
- The two gating softmaxes run over the sequence axis (dim=1). Terms constant
  along that axis cancel in softmax exactly: d_theta_b and th1 @ d_theta_w[:H]
  (diff branch) and v_gamma_b (van branch) are all dead. This kills the whole
  th1/bias1 chain and the WD_w / d_theta_w[:H] weights.
- Sharding: rows of the flattened (B*S, H) activations, 256 per core; cores
  0-3 own batch 0, 4-7 batch 1. Each core redundantly computes full-batch K/V
  (collective reshards measure slower than the redundant GEMMs on this part).
- Attention head pairs are packed into disjoint PE row groups (K=64 each); the
  two scores matmuls of a pair write disjoint halves of one [128,512] PSUM
  bank and a single exp ACTIVATE covers both heads.
- The two sequence-axis softmax denominators are summed across the 4-core
  batch group with small AllGathers + local adds. The gathers are split into
  trigger (right after the partial sums) and finish (after the filler GEMMs),
  and every DMA needed before a gather resolves is enqueued ahead of it: the
  sync DMA queue is strictly in-order, so a descriptor gated on the collective
  would head-of-line block all later weight loads (measured 26us PE stall).
- Weight loads use one dma_start per tile (the sync queue costs ~600ns per
  entry) and are prefetched in need-order; the order is chosen so weight-pool
  slot recycling (bufs=5) never gates a DMA on a reader that runs later.
- Compute in bf16 with fp32 accumulation; exp/tanh/sigmoid on ACT; identity
  epilogues on DVE to keep ACT for transcendentals.
"""

import numpy as np
import ml_dtypes

import concourse.bass as bass
import concourse.mybir as mybir
import concourse.tile as tile
from concourse import bacc
from concourse.bass_utils import run_bass_kernel_spmd

B, S, H = 2, 1024, 768
NH, DH = 12, 64
P = 128
RV = 256            # rows per core
KC = H // P         # 6 channel chunks
JC = S // P         # 8 sequence chunks
GROUPS = [[0, 1, 2, 3], [4, 5, 6, 7]]
SCALE = 1.0 / 8.0   # 1/sqrt(DH)

bf16 = mybir.dt.bfloat16
f32 = mybir.dt.float32
AF = mybir.ActivationFunctionType
ALU = mybir.AluOpType
nbf16 = ml_dtypes.bfloat16

W768 = ["vq_w", "vk_w", "vv_w", "dv_w", "van_fc_w", "WV_w", "diff_fc_w",
        "diff_fus_w", "van_fus_w", "nf_w", "final_w"]
W1536 = ["d_theta_w", "v_gamma_w", "diff_out_w", "van_out_w"]
BIAS = ["vq_b", "vk_b", "dv_b", "van_fc_b", "diff_fc_b",
        "diff_out_b", "van_out_b", "diff_fus_b", "van_fus_b",
        "nf_b", "final_b"]


def build(has_vvb: bool):
    nc = bacc.Bacc(None, target_bir_lowering=False, debug=False, num_devices=8)

    xT_d = nc.dram_tensor("xT", [H, RV], bf16, kind="ExternalInput")
    yT_d = nc.dram_tensor("yT", [H, S], bf16, kind="ExternalInput")
    wd = {}
    for w in W768:
        wd[w] = nc.dram_tensor(w, [H, H], bf16, kind="ExternalInput")
    for w in W1536:
        wd[w] = nc.dram_tensor(w, [2 * H, H], bf16, kind="ExternalInput")
    wd["gate_w"] = nc.dram_tensor("gate_w", [2 * H, 1], bf16, kind="ExternalInput")
    wd["nf_out_w"] = nc.dram_tensor("nf_out_w", [2 * H, 1], bf16, kind="ExternalInput")
    bd = {}
    for b in BIAS:
        bd[b] = nc.dram_tensor(b, [H], f32, kind="ExternalInput")
    if has_vvb:
        bd["vv_b"] = nc.dram_tensor("vv_b", [H], f32, kind="ExternalInput")
    out_d = nc.dram_tensor("outT", [H, RV], f32, kind="ExternalOutput")

    with tile.TileContext(nc, num_cores=8) as tc:
        with (
            tc.tile_pool(name="wpool", bufs=6) as wp,
            tc.tile_pool(name="wsmall", bufs=2) as wsp,
            tc.tile_pool(name="acts", bufs=1) as ap,
            tc.tile_pool(name="loop", bufs=2) as lp,
            tc.tile_pool(name="psum", bufs=8, space="PSUM") as pp,
            tc.tile_pool(name="dram", bufs=1, space="DRAM") as dp,
        ):
            def wtile(name, half=None, split=False):
                t = wp.tile([P, KC, H], bf16, name=f"w_{name}_{half}", tag="w")
                src = wd[name]
                if half is not None:
                    src = src[half * H:(half + 1) * H, :]
                src = src.rearrange("(kc p) n -> kc p n", p=P)
                for kc in range(KC):
                    nc.sync.dma_start(t[:, kc, :], src[kc])
                return t

            def btile(name):
                t = ap.tile([P, KC], f32, name=f"b_{name}")
                nc.sync.dma_start(t[:], bd[name].rearrange("(c p) -> p c", p=P))
                return t

            def brow(name):
                t = ap.tile([1, H], f32, name=f"br_{name}")
                nc.sync.dma_start(t[:], bd[name].rearrange("(o c) -> o c", o=1))
                return t

            # ---------------- Q projection first: minimal-dependency PE work
            b_vq = btile("vq_b")
            xT = ap.tile([P, KC, RV], bf16, name="xT")
            for kc in range(KC):
                nc.sync.dma_start(xT[:, kc, :], xT_d.rearrange(
                    "(kc p) n -> kc p n", p=P)[kc])
            w_vq = wtile("vq_w")
            b_vk = btile("vk_b")
            w_vk = wtile("vk_w")
            yT = ap.tile([P, KC, S], bf16, name="yT")
            for kc in range(KC):
                nc.sync.dma_start(yT[:, kc, :], yT_d.rearrange(
                    "(kc p) n -> kc p n", p=P)[kc])
            w_vv = wtile("vv_w")

            # warm up the collective stream early: the first real collective
            # otherwise pays an ~24us trigger-start delay.
            dgi = dp.tile([P, 1], f32, name="dgi")
            dgo = dp.tile([4 * P, 1], f32, name="dgo")
            nc.sync.dma_start(dgi[:], b_vq[:, 0:1])
            nc.gpsimd.collective_compute(
                "AllGather", ALU.bypass, replica_groups=GROUPS,
                ins=[dgi[:]], outs=[dgo[:]])

            qT = ap.tile([P, KC, RV], bf16, name="qT")
            for mc in range(KC):
                ps = pp.tile([P, RV], f32, name=f"qps{mc}", tag="big", bufs=3)
                for kc in range(KC):
                    nc.tensor.matmul(ps[:], w_vq[:, kc, mc * P:(mc + 1) * P],
                                     xT[:, kc, :],
                                     start=(kc == 0), stop=(kc == KC - 1))
                nc.vector.tensor_scalar_add(qT[:, mc, :], ps[:],
                                            b_vq[:, mc:mc + 1])

            ones64 = ap.tile([1, 64], bf16, name="ones64")
            nc.vector.memset(ones64[:], 1.0)
            ones128 = ap.tile([1, P], bf16, name="ones128")
            nc.vector.memset(ones128[:], 1.0)

            # ---------------- K/V projections, emitted in two halves so the
            # attention pairs of the first half overlap the second half ------
            kT = ap.tile([P, KC, S], bf16, name="kT")
            v_aug = ap.tile([P, JC, NH, DH + 1], bf16, name="v_aug")
            nc.vector.memset(v_aug[:, :, :, DH:DH + 1], 1.0)

            # Epilogues of k/v projections run on GPSIMD (otherwise idle):
            # the scores/PV matmuls gate on these, and DVE was measured as
            # the attention-phase serializer (17.8us of PE stalls).
            def kproj(mc):
                for sh in range(2):
                    ps = pp.tile([P, 512], f32, name=f"kps{mc}_{sh}",
                                 tag="big", bufs=3)
                    for kc in range(KC):
                        nc.tensor.matmul(
                            ps[:], w_vk[:, kc, mc * P:(mc + 1) * P],
                            yT[:, kc, sh * 512:(sh + 1) * 512],
                            start=(kc == 0), stop=(kc == KC - 1))
                    nc.scalar.activation(
                        kT[:, mc, sh * 512:(sh + 1) * 512], ps[:],
                        AF.Identity, bias=b_vk[:, mc:mc + 1])

            def vproj(cg):
                for jc in range(JC):
                    ps = pp.tile([P, 384], f32, name=f"vps{jc}_{cg}",
                                 tag="big", bufs=3)
                    for kc in range(KC):
                        nc.tensor.matmul(
                            ps[:], yT[:, kc, jc * P:(jc + 1) * P],
                            w_vv[:, kc, cg * 384:(cg + 1) * 384],
                            start=(kc == 0), stop=(kc == KC - 1))
                    nc.vector.tensor_copy(
                        v_aug[:, jc, cg * 6:(cg + 1) * 6, 0:DH],
                        ps[:].rearrange("p (h d) -> p h d", d=DH))

            # ---------------- per-batch chain pieces ------------------------
            yb = ap.tile([P, KC], f32, name="yb")
            for kc in range(KC):
                nc.vector.tensor_reduce(yb[:, kc:kc + 1], yT[:, kc, :],
                                        axis=mybir.AxisListType.X, op=ALU.add)
            ybt = ap.tile([P, KC], bf16, name="ybt")
            nc.vector.tensor_scalar_mul(ybt[:], yb[:], 1.0 / S)

            def vchain_cm(vec_cm, w_t, bias_cm, name):
                # chan-major out [128, 6] f32 = vec @ W + bias, computed
                # directly in chan-major form (stationary = weight chunk,
                # moving = the [128,1] chan-major vector chunk): no DRAM
                # bounce, so the sync DMA queue never blocks on PE progress.
                out = ap.tile([P, KC], f32, name=f"{name}_cm32")
                for mc in range(KC):
                    ps = pp.tile([P, 1], f32, name=f"{name}ps{mc}",
                                 tag="sps", bufs=2)
                    for kc in range(KC):
                        nc.tensor.matmul(ps[:],
                                         w_t[:, kc, mc * P:(mc + 1) * P],
                                         vec_cm[:, kc:kc + 1],
                                         start=(kc == 0), stop=(kc == KC - 1))
                    if bias_cm is not None:
                        nc.vector.tensor_add(out[:, mc:mc + 1], ps[:],
                                             bias_cm[:, mc:mc + 1])
                    else:
                        nc.vector.tensor_copy(out[:, mc:mc + 1], ps[:])
                return out

            # ---------------- attention: pairs pipelined against K/V -------
            w_dv = wtile("dv_w")
            b_dv = btile("dv_b")
            w_do0 = wtile("diff_out_w", half=0)
            b_dout = btile("diff_out_b")
            if has_vvb:
                b_vv = btile("vv_b")
            vanT = ap.tile([P, KC, RV], bf16, name="vanT")
            acc_t2 = ap.tile([P, KC, RV], f32, name="acc_t2")

            def van_partial(kc, w_t, acc, tg):
                # acc += w[kc-chunk].T @ vanT[kc] as soon as the pair-kc
                # attention output lands; accumulate in SBUF f32 on GPSIMD.
                for mc in range(KC):
                    ps = pp.tile([P, RV], f32, name=f"{tg}p{kc}_{mc}",
                                 tag="pv", bufs=3)
                    nc.tensor.matmul(ps[:], w_t[:, kc, mc * P:(mc + 1) * P],
                                     vanT[:, kc, :], start=True, stop=True)
                    if kc == 0:
                        nc.vector.tensor_copy(acc[:, mc, :], ps[:])
                    else:
                        nc.vector.tensor_add(acc[:, mc, :], acc[:, mc, :],
                                             ps[:])

            def pair_tail(hq, pvs):
                hp = hq
                for hh in range(2):
                    invZb = lp.tile([1, RV], bf16, name=f"invZb{2 * hq + hh}",
                                    tag="invZb")
                    with nc.allow_low_precision(reason="1/Z feeds bf16 mul"):
                        nc.vector.reciprocal(invZb[:], pvs[hh][DH:DH + 1, :])
                    bc = pp.tile([64, RV], f32, name=f"bc{2 * hq + hh}",
                                 tag="big", bufs=3)
                    nc.tensor.matmul(bc[:], ones64[:], invZb[:],
                                     start=True, stop=True)
                    bcs = lp.tile([64, RV], bf16, name=f"bcs{2 * hq + hh}",
                                  tag="bcs")
                    nc.vector.tensor_copy(bcs[:], bc[:])
                    nc.vector.tensor_mul(vanT[hh * 64:hh * 64 + 64, hp, :],
                                         pvs[hh][0:DH, :], bcs[:])
                    if has_vvb:
                        nc.vector.tensor_scalar_add(
                            vanT[hh * 64:hh * 64 + 64, hp, :],
                            vanT[hh * 64:hh * 64 + 64, hp, :],
                            b_vv[hh * 64:hh * 64 + 64, hp:hp + 1])

            def pair_block(hp, prev):
                # scores+exp for pair hp, with the PV matmuls of the previous
                # pair interleaved into the same jc loop so the PE never
                # stalls on ACT's exp backlog. Both heads of the pair share
                # one [128,512] PSUM bank -> one exp ACTIVATE per block.
                e = lp.tile([P, JC, 512], bf16, name=f"e{hp}", tag="expT",
                            bufs=3)
                if prev is not None:
                    hq, eq = prev
                    pvs = [pp.tile([DH + 1, RV], f32, name=f"pv{2 * hq + hh}",
                                   tag="pv", bufs=3) for hh in range(2)]
                for jc in range(JC):
                    for hh in range(2):
                        lo = hh * 64
                        sc = pp.tile([P, RV], f32, name=f"sc{hp}_{jc}_{hh}",
                                     tag="big", bufs=3)
                        nc.tensor.matmul(
                            sc[:],
                            kT[lo:lo + 64, hp, jc * P:(jc + 1) * P],
                            qT[lo:lo + 64, hp, :],
                            start=True, stop=True)
                        nc.scalar.activation(
                            e[:, jc, hh * RV:(hh + 1) * RV], sc[:],
                            AF.Exp, scale=SCALE)
                    if prev is not None:
                        for hh in range(2):
                            nc.tensor.matmul(
                                pvs[hh][:], v_aug[:, jc, 2 * hq + hh, :],
                                eq[:, jc, hh * RV:(hh + 1) * RV],
                                start=(jc == 0), stop=(jc == JC - 1))
                if prev is not None:
                    pair_tail(hq, pvs)
                return e

            def last_pv(hq, eq):
                pvs = [pp.tile([DH + 1, RV], f32, name=f"pv{2 * hq + hh}",
                               tag="pv", bufs=3) for hh in range(2)]
                for jc in range(JC):
                    for hh in range(2):
                        nc.tensor.matmul(
                            pvs[hh][:], v_aug[:, jc, 2 * hq + hh, :],
                            eq[:, jc, hh * RV:(hh + 1) * RV],
                            start=(jc == 0), stop=(jc == JC - 1))
                pair_tail(hq, pvs)

            kproj(0)
            kproj(1)
            kproj(2)
            vproj(0)
            e0 = pair_block(0, None)
            m32 = vchain_cm(ybt, w_dv, b_dv, "m")
            m_cm = ap.tile([P, KC], bf16, name="m_cm")
            nc.vector.tensor_copy(m_cm[:], m32[:])
            e1h = pair_block(1, (0, e0))
            e2h = pair_block(2, (1, e1h))
            w_vfc = wtile("van_fc_w")
            b_vfc = btile("van_fc_b")
            van_partial(0, w_vfc, acc_t2, "t2")
            kproj(3)
            kproj(4)
            kproj(5)
            vproj(1)
            # gating-weight prefetch: issued mid-attention in need-order.
            # With bufs=6 the slot gates (6-back readers) all resolve at or
            # before each weight's emission point, so the in-order sync DMA
            # queue never blocks ahead of the z1 trigger.
            w_dth1 = wtile("d_theta_w", half=1)
            w_WV = wtile("WV_w")
            w_vg0 = wtile("v_gamma_w", half=0)
            w_vo0 = wtile("van_out_w", half=0)
            b_vo = btile("van_out_b")
            b_dfc = btile("diff_fc_b")
            ws_gate = wsp.tile([P, 2 * KC, 1], bf16, name="ws_gate", tag="ws")
            nc.sync.dma_start(ws_gate[:], wd["gate_w"].rearrange(
                "(c p) o -> p c o", p=P))
            ws_nf = wsp.tile([P, 2 * KC, 1], bf16, name="ws_nf", tag="ws")
            nc.sync.dma_start(ws_nf[:], wd["nf_out_w"].rearrange(
                "(c p) o -> p c o", p=P))
            van_partial(1, w_vfc, acc_t2, "t2")
            w_dfc = wtile("diff_fc_w")
            e3h = pair_block(3, (2, e2h))
            b_dfus = btile("diff_fus_b")
            b_vfus = btile("van_fus_b")
            b_nf = btile("nf_b")
            b_fin = btile("final_b")
            e4h = pair_block(4, (3, e3h))
            van_partial(2, w_vfc, acc_t2, "t2")
            e5h = pair_block(5, (4, e4h))
            van_partial(3, w_vfc, acc_t2, "t2")
            # van_partial(4) reads vanT[4] written by pair_tail inside the
            # pair-5 block just emitted; the bias2 vchain gives the DVE chain
            # time to land before the PE reaches it.
            bias2 = vchain_cm(m_cm, w_do0, b_dout, "bias2")
            van_partial(4, w_vfc, acc_t2, "t2")
            last_pv(5, e5h)
            van_partial(5, w_vfc, acc_t2, "t2")
            # vg1 slot-gates on van_partial(5) (vfc's last read): emit here so
            # the queue unblocks right away; it lands during the e1 gemm.
            w_vg1 = wtile("v_gamma_w", half=1)

            # ---------------- gating network --------------------------------
            def gemm(pairs, func, bias_t=None, accum_t=None, name="g",
                     out_dt=bf16, pre=None):
                out = ap.tile([P, KC, RV], out_dt, name=name)
                nmm = len(pairs) * KC
                for mc in range(KC):
                    ps = pp.tile([P, RV], f32, name=f"{name}ps{mc}", tag="big",
                                 bufs=3)
                    i = 0
                    for wt, at in pairs:
                        for kc in range(KC):
                            nc.tensor.matmul(ps[:],
                                             wt[:, kc, mc * P:(mc + 1) * P],
                                             at[:, kc, :],
                                             start=(i == 0), stop=(i == nmm - 1))
                            i += 1
                    src = ps
                    if pre is not None:
                        tmp = lp.tile([P, RV], f32, name=f"{name}pre{mc}",
                                      tag="pretmp")
                        nc.vector.tensor_add(tmp[:], ps[:], pre[:, mc, :])
                        src = tmp
                    if func == AF.Identity and accum_t is None:
                        if bias_t is not None:
                            nc.vector.tensor_scalar_add(out[:, mc, :], src[:],
                                                        bias_t[:, mc:mc + 1])
                        else:
                            nc.vector.tensor_copy(out[:, mc, :], src[:])
                    else:
                        nc.scalar.activation(
                            out[:, mc, :], src[:], func,
                            bias=(bias_t[:, mc:mc + 1] if bias_t is not None
                                  else 0.0),
                            accum_out=(accum_t[:, mc:mc + 1]
                                       if accum_t is not None else None))
                return out

            def ag_start(part, name):
                gi = dp.tile([P, KC], f32, name=f"gi_{name}")
                go = dp.tile([4 * P, KC], f32, name=f"go_{name}")
                nc.sync.dma_start(gi[:], part[:])
                nc.gpsimd.collective_compute(
                    "AllGather", ALU.bypass, replica_groups=GROUPS,
                    ins=[gi[:]], outs=[go[:]])
                return go

            def ag_finish(go, name):
                zt = ap.tile([P, 4, KC], f32, name=f"zt_{name}")
                nc.sync.dma_start(zt[:], go.rearrange("(r p) c -> p r c", p=P))
                z = ap.tile([P, KC], f32, name=f"z_{name}")
                nc.vector.tensor_add(z[:], zt[:, 0, :], zt[:, 1, :])
                nc.vector.tensor_add(z[:], z[:], zt[:, 2, :])
                nc.vector.tensor_add(z[:], z[:], zt[:, 3, :])
                return z

            theta2 = ap.tile([P, KC, RV], bf16, name="theta2")
            for mc in range(KC):
                nc.scalar.activation(theta2[:, mc, :], acc_t2[:, mc, :],
                                     AF.Tanh, bias=b_vfc[:, mc:mc + 1])

            part1 = ap.tile([P, KC], f32, name="part1")
            e1 = gemm([(w_dth1, theta2)], AF.Exp, accum_t=part1, name="e1")
            go1 = ag_start(part1, "z1")
            # readback enqueued right behind the trigger: anything between
            # them would delay the gather result by ~600ns per queue entry.
            z1 = ag_finish(go1, "z1")

            # --- AllGather-1 bubble fillers (independent of z1) -------------
            # gamma1 and voa interleaved at the mc level so each one's ACT
            # epilogues hide under the other's matmuls. Weight DMAs for the
            # post-z1 GEMMs are emitted here in need-order; each slot gate
            # resolves no later than the previous one (monotone), so the
            # queue drains without head-of-line blocking.
            w_do1 = wtile("diff_out_w", half=1)
            gamma1 = ap.tile([P, KC, RV], bf16, name="gamma1")
            voa = ap.tile([P, KC, RV], f32, name="voa")
            for mc in range(KC):
                ps1 = pp.tile([P, RV], f32, name=f"g1ps{mc}", tag="big",
                              bufs=3)
                for kc in range(KC):
                    nc.tensor.matmul(ps1[:], w_WV[:, kc, mc * P:(mc + 1) * P],
                                     vanT[:, kc, :],
                                     start=(kc == 0), stop=(kc == KC - 1))
                nc.scalar.activation(gamma1[:, mc, :], ps1[:], AF.Tanh)
                ps2 = pp.tile([P, RV], f32, name=f"voaps{mc}", tag="big",
                              bufs=3)
                for kc in range(KC):
                    nc.tensor.matmul(ps2[:], w_vo0[:, kc, mc * P:(mc + 1) * P],
                                     vanT[:, kc, :],
                                     start=(kc == 0), stop=(kc == KC - 1))
                nc.vector.tensor_scalar_add(voa[:, mc, :], ps2[:],
                                            b_vo[:, mc:mc + 1])
            w_dfus = wtile("diff_fus_w")
            z2a = gemm([(w_vg0, gamma1)], AF.Identity, name="z2a", out_dt=f32)
            w_vo1 = wtile("van_out_w", half=1)
            ps_nf = pp.tile([1, RV], f32, name="nfps", tag="sps", bufs=2)
            for kc in range(KC):
                nc.tensor.matmul(ps_nf[:], ws_nf[:, kc, :], vanT[:, kc, :],
                                 start=(kc == 0), stop=False,
                                 skip_group_check=True)
            w_vfus = wtile("van_fus_w")

            s1 = ap.tile([P, KC], f32, name="s1")
            nc.vector.reciprocal(s1[:], z1[:])
            nc.vector.tensor_mul(s1[:], s1[:], m32[:])
            dth = ap.tile([P, KC, RV], bf16, name="dth")
            for mc in range(KC):
                nc.vector.tensor_scalar_mul(dth[:, mc, :], e1[:, mc, :],
                                            s1[:, mc:mc + 1])

            gamma2 = gemm([(w_dfc, dth)], AF.Tanh, bias_t=b_dfc, name="gamma2")
            w_nf = wtile("nf_w")
            part2 = ap.tile([P, KC], f32, name="part2")
            e2 = gemm([(w_vg1, gamma2)], AF.Exp, accum_t=part2, pre=z2a,
                      name="e2")
            go2 = ag_start(part2, "z2")
            z2 = ag_finish(go2, "z2")
            w_fin = wtile("final_w")

            # --- AllGather-2 bubble fillers --------------------------------
            dout = gemm([(w_do1, dth)], AF.Tanh, bias_t=bias2, name="dout")
            dfus = gemm([(w_dfus, dout)], AF.Tanh, bias_t=b_dfus, name="dfus")

            s2 = ap.tile([P, KC], f32, name="s2")
            nc.vector.reciprocal(s2[:], z2[:])
            ag = ap.tile([P, KC, RV], bf16, name="ag")
            for mc in range(KC):
                nc.vector.scalar_tensor_tensor(
                    ag[:, mc, :], e2[:, mc, :], s2[:, mc:mc + 1],
                    vanT[:, mc, :], op0=ALU.mult, op1=ALU.mult)

            vout = gemm([(w_vo1, ag)], AF.Tanh, pre=voa, name="vout")
            vfus = gemm([(w_vfus, vout)], AF.Tanh, bias_t=b_vfus, name="vfus")
            diffv = ap.tile([P, KC, RV], bf16, name="diffv")
            for mc in range(KC):
                nc.vector.tensor_sub(diffv[:, mc, :], vfus[:, mc, :],
                                     dfus[:, mc, :])

            # gate (M=1 GEMM over both fusion tensors)
            ps_g = pp.tile([1, RV], f32, name="gateps", tag="sps", bufs=2)
            i = 0
            for at, base in [(dfus, 0), (vfus, KC)]:
                for kc in range(KC):
                    nc.tensor.matmul(ps_g[:], ws_gate[:, base + kc, :],
                                     at[:, kc, :],
                                     start=(i == 0), stop=(i == 2 * KC - 1))
                    i += 1
            gb16 = ap.tile([1, RV], bf16, name="gb16")
            nc.scalar.activation(gb16[:], ps_g[:], AF.Sigmoid)
            gbc = pp.tile([P, RV], f32, name="gbc", tag="pv", bufs=3)
            nc.tensor.matmul(gbc[:], ones128[:], gb16[:], start=True, stop=True)

            fus = ap.tile([P, KC, RV], bf16, name="fus")
            for mc in range(KC):
                t2 = lp.tile([P, RV], bf16, name=f"ft2_{mc}", tag="ft2")
                nc.vector.tensor_mul(t2[:], diffv[:, mc, :], gbc[:])
                nc.vector.tensor_add(fus[:, mc, :], t2[:], dfus[:, mc, :])

            # tnf first, so the nf sigmoid/broadcast is ready before the
            # final tanh GEMM and its fused mul+store epilogue.
            tnf = gemm([(w_nf, fus)], AF.Identity, bias_t=b_nf, name="tnf")
            for kc in range(KC):
                nc.tensor.matmul(ps_nf[:], ws_nf[:, KC + kc, :], tnf[:, kc, :],
                                 start=False, stop=(kc == KC - 1),
                                 skip_group_check=True)
            nb16 = ap.tile([1, RV], bf16, name="nb16")
            nc.scalar.activation(nb16[:], ps_nf[:], AF.Sigmoid)
            nbc = pp.tile([P, RV], f32, name="nbc", tag="pv", bufs=3)
            nc.tensor.matmul(nbc[:], ones128[:], nb16[:], start=True, stop=True)

            od = out_d.rearrange("(mc p) n -> mc p n", p=P)
            for mc in range(KC):
                ps = pp.tile([P, RV], f32, name=f"ftps{mc}", tag="big", bufs=3)
                for kc in range(KC):
                    nc.tensor.matmul(ps[:], w_fin[:, kc, mc * P:(mc + 1) * P],
                                     fus[:, kc, :],
                                     start=(kc == 0), stop=(kc == KC - 1))
                ftc = lp.tile([P, RV], bf16, name=f"ftc{mc}", tag="ftc")
                nc.scalar.activation(ftc[:], ps[:], AF.Tanh,
                                     bias=b_fin[:, mc:mc + 1])
                ot = lp.tile([P, RV], f32, name=f"ot{mc}", tag="ot", bufs=3)
                nc.vector.tensor_mul(ot[:], ftc[:], nbc[:])
                nc.sync.dma_start(od[mc], ot[:])

    nc.compile()
    return nc


_CACHE = {}


def _prep_in_maps(inputs):
    x = np.asarray(inputs["x"], np.float32)
    y = np.asarray(inputs["y"], np.float32)
    has_vvb = bool(np.any(np.asarray(inputs["vv_b"]) != 0))

    xt = np.ascontiguousarray(x.reshape(B * S, H).T).astype(nbf16)   # [H, 2048]
    yts = [np.ascontiguousarray(y[b].T).astype(nbf16) for b in range(B)]

    base = {}
    for w in W768 + W1536 + ["gate_w", "nf_out_w"]:
        base[w] = np.asarray(inputs[w], np.float32).astype(nbf16)
    for b in BIAS:
        base[b] = np.ascontiguousarray(np.asarray(inputs[b], np.float32))
    if has_vvb:
        base["vv_b"] = np.ascontiguousarray(np.asarray(inputs["vv_b"], np.float32))

    in_maps = []
    for c in range(8):
        bat = c // 4
        m = dict(base)
        m["xT"] = np.ascontiguousarray(xt[:, c * RV:(c + 1) * RV])
        m["yT"] = yts[bat]
        in_maps.append(m)
    return in_maps, has_vvb


def kernel(**inputs):
    in_maps, has_vvb = _prep_in_maps(inputs)
    if has_vvb not in _CACHE:
        _CACHE[has_vvb] = build(has_vvb)
    nc = _CACHE[has_vvb]

    res = run_bass_kernel_spmd(nc, in_maps, core_ids=list(range(8)))
    full = np.concatenate([res.results[c]["outT"] for c in range(8)], axis=1)
    return np.ascontiguousarray(full.T.reshape(B, S, H)).astype(np.float32)


if __name__ == "__main__":
    rng = np.random.default_rng(0)
    ins = {"x": rng.standard_normal((B, S, H)).astype(np.float32),
           "y": rng.standard_normal((B, S, H)).astype(np.float32)}
    for w in W768 + W1536 + ["dq_w", "dk_w", "WD_w"]:
        shp = (H, H) if w not in W1536 else (2 * H, H)
        ins[w] = (rng.standard_normal(shp) * 0.02).astype(np.float32)
    ins["gate_w"] = (rng.standard_normal((2 * H, 1)) * 0.02).astype(np.float32)
    ins["nf_out_w"] = (rng.standard_normal((2 * H, 1)) * 0.02).astype(np.float32)
    for b in BIAS + ["vv_b", "dq_b", "dk_b", "d_theta_b", "v_gamma_b"]:
        ins[b] = np.zeros(H, np.float32)
    out = kernel(**ins)
    print("out", out.shape, out.dtype, np.abs(out).mean())


# revision 47
# speedup vs baseline: 1.1825x; 1.0365x over previous
"""Coupled-attention module as a distributed Bass/Tile kernel on 8 TRN2 cores.

Math notes (exact algebra, not approximations):
- The differential-attention scores are constant along the softmax axis, so
  softmax yields exactly uniform 1/S weights: diff_vector collapses to the
  per-batch mean of (y @ dv_w + dv_b), broadcast over sequence. dq/dk are dead.
- The two gating softmaxes run over the sequence axis (dim=1). Terms constant
  along that axis cancel in softmax exactly: d_theta_b and th1 @ d_theta_w[:H]
  (diff branch) and v_gamma_b (van branch) are all dead. This kills the whole
  th1/bias1 chain and the WD_w / d_theta_w[:H] weights.
- Sharding: rows of the flattened (B*S, H) activations, 256 per core; cores
  0-3 own batch 0, 4-7 batch 1. Each core redundantly computes full-batch K/V
  (collective reshards measure slower than the redundant GEMMs on this part).
- Attention head pairs are packed into disjoint PE row groups (K=64 each); the
  two scores matmuls of a pair write disjoint halves of one [128,512] PSUM
  bank and a single exp ACTIVATE covers both heads.
- The two sequence-axis softmax denominators are summed across the 4-core
  batch group with small AllGathers + local adds. The gathers are split into
  trigger (right after the partial sums) and finish (after the filler GEMMs),
  and every DMA needed before a gather resolves is enqueued ahead of it: the
  sync DMA queue is strictly in-order, so a descriptor gated on the collective
  would head-of-line block all later weight loads (measured 26us PE stall).
- Weight loads use one dma_start per tile (the sync queue costs ~600ns per
  entry) and are prefetched in need-order; the order is chosen so weight-pool
  slot recycling (bufs=5) never gates a DMA on a reader that runs later.
- Compute in bf16 with fp32 accumulation; exp/tanh/sigmoid on ACT; identity
  epilogues on DVE to keep ACT for transcendentals.
"""

import numpy as np
import ml_dtypes

import concourse.bass as bass
import concourse.mybir as mybir
import concourse.tile as tile
from concourse import bacc
from concourse.bass_utils import run_bass_kernel_spmd

B, S, H = 2, 1024, 768
NH, DH = 12, 64
P = 128
RV = 256            # rows per core
KC = H // P         # 6 channel chunks
JC = S // P         # 8 sequence chunks
GROUPS = [[0, 1, 2, 3], [4, 5, 6, 7]]
SCALE = 1.0 / 8.0   # 1/sqrt(DH)

bf16 = mybir.dt.bfloat16
f32 = mybir.dt.float32
AF = mybir.ActivationFunctionType
ALU = mybir.AluOpType
nbf16 = ml_dtypes.bfloat16

W768 = ["vq_w", "vk_w", "vv_w", "dv_w", "van_fc_w", "WV_w", "diff_fc_w",
        "diff_fus_w", "van_fus_w", "nf_w", "final_w"]
W1536 = ["d_theta_w", "v_gamma_w", "diff_out_w", "van_out_w"]
BIAS = ["vq_b", "vk_b", "dv_b", "van_fc_b", "diff_fc_b",
        "diff_out_b", "van_out_b", "diff_fus_b", "van_fus_b",
        "nf_b", "final_b"]


def build(has_vvb: bool):
    nc = bacc.Bacc(None, target_bir_lowering=False, debug=False, num_devices=8)

    # All inputs are pre-transposed on the host into partition-major layouts
    # so every load is ONE contiguous 2-D DMA: the in-order sync DMA queue
    # costs ~650ns of issue time per dma_start, and per-chunk loads (6 per
    # weight) were measured delaying the z1 collective trigger by ~7us.
    xT_d = nc.dram_tensor("xT", [P, KC * RV], bf16, kind="ExternalInput")
    yT_d = nc.dram_tensor("yT", [P, KC * S], bf16, kind="ExternalInput")
    wd = {}
    for w in W768:
        wd[w] = nc.dram_tensor(w, [P, KC * H], bf16, kind="ExternalInput")
    for w in W1536:
        wd[w] = nc.dram_tensor(w, [P, 2 * KC * H], bf16, kind="ExternalInput")
    wd["gate_w"] = nc.dram_tensor("gate_w", [P, 2 * KC], bf16, kind="ExternalInput")
    wd["nf_out_w"] = nc.dram_tensor("nf_out_w", [P, 2 * KC], bf16, kind="ExternalInput")
    bd = {}
    for b in BIAS:
        bd[b] = nc.dram_tensor(b, [P, KC], f32, kind="ExternalInput")
    if has_vvb:
        bd["vv_b"] = nc.dram_tensor("vv_b", [P, KC], f32, kind="ExternalInput")
    out_d = nc.dram_tensor("outT", [H, RV], f32, kind="ExternalOutput")

    with tile.TileContext(nc, num_cores=8) as tc:
        with (
            tc.tile_pool(name="wpool", bufs=6) as wp,
            tc.tile_pool(name="wsmall", bufs=2) as wsp,
            tc.tile_pool(name="acts", bufs=1) as ap,
            tc.tile_pool(name="loop", bufs=2) as lp,
            tc.tile_pool(name="psum", bufs=8, space="PSUM") as pp,
            tc.tile_pool(name="dram", bufs=1, space="DRAM") as dp,
        ):
            def wtile(name, half=None):
                t = wp.tile([P, KC, H], bf16, name=f"w_{name}_{half}", tag="w")
                if half is not None:
                    src = wd[name][:, half * KC * H:(half + 1) * KC * H]
                else:
                    src = wd[name][:]
                nc.sync.dma_start(t[:].rearrange("p a b -> p (a b)"), src)
                return t

            def btile(name):
                t = ap.tile([P, KC], f32, name=f"b_{name}")
                nc.sync.dma_start(t[:], bd[name][:])
                return t

            # ---------------- Q projection first: minimal-dependency PE work
            b_vq = btile("vq_b")
            xT = ap.tile([P, KC, RV], bf16, name="xT")
            nc.sync.dma_start(xT[:].rearrange("p a b -> p (a b)"), xT_d[:])
            w_vq = wtile("vq_w")
            b_vk = btile("vk_b")
            w_vk = wtile("vk_w")
            yT = ap.tile([P, KC, S], bf16, name="yT")
            nc.sync.dma_start(yT[:].rearrange("p a b -> p (a b)"), yT_d[:])
            w_vv = wtile("vv_w")

            # warm up the collective stream early: the first real collective
            # otherwise pays an ~24us trigger-start delay.
            dgi = dp.tile([P, 1], f32, name="dgi")
            dgo = dp.tile([4 * P, 1], f32, name="dgo")
            nc.sync.dma_start(dgi[:], b_vq[:, 0:1])
            nc.gpsimd.collective_compute(
                "AllGather", ALU.bypass, replica_groups=GROUPS,
                ins=[dgi[:]], outs=[dgo[:]])

            qT = ap.tile([P, KC, RV], bf16, name="qT")
            for mc in range(KC):
                ps = pp.tile([P, RV], f32, name=f"qps{mc}", tag="big", bufs=3)
                for kc in range(KC):
                    nc.tensor.matmul(ps[:], w_vq[:, kc, mc * P:(mc + 1) * P],
                                     xT[:, kc, :],
                                     start=(kc == 0), stop=(kc == KC - 1))
                nc.vector.tensor_scalar_add(qT[:, mc, :], ps[:],
                                            b_vq[:, mc:mc + 1])

            ones64 = ap.tile([1, 64], bf16, name="ones64")
            nc.vector.memset(ones64[:], 1.0)
            ones128 = ap.tile([1, P], bf16, name="ones128")
            nc.vector.memset(ones128[:], 1.0)

            # ---------------- K/V projections, emitted in two halves so the
            # attention pairs of the first half overlap the second half ------
            kT = ap.tile([P, KC, S], bf16, name="kT")
            v_aug = ap.tile([P, JC, NH, DH + 1], bf16, name="v_aug")
            nc.vector.memset(v_aug[:, :, :, DH:DH + 1], 1.0)

            # Epilogues of k/v projections run on GPSIMD (otherwise idle):
            # the scores/PV matmuls gate on these, and DVE was measured as
            # the attention-phase serializer (17.8us of PE stalls).
            def kproj(mc):
                for sh in range(2):
                    ps = pp.tile([P, 512], f32, name=f"kps{mc}_{sh}",
                                 tag="big", bufs=3)
                    for kc in range(KC):
                        nc.tensor.matmul(
                            ps[:], w_vk[:, kc, mc * P:(mc + 1) * P],
                            yT[:, kc, sh * 512:(sh + 1) * 512],
                            start=(kc == 0), stop=(kc == KC - 1))
                    nc.scalar.activation(
                        kT[:, mc, sh * 512:(sh + 1) * 512], ps[:],
                        AF.Identity, bias=b_vk[:, mc:mc + 1])

            def vproj(cg):
                for jc in range(JC):
                    ps = pp.tile([P, 384], f32, name=f"vps{jc}_{cg}",
                                 tag="big", bufs=3)
                    for kc in range(KC):
                        nc.tensor.matmul(
                            ps[:], yT[:, kc, jc * P:(jc + 1) * P],
                            w_vv[:, kc, cg * 384:(cg + 1) * 384],
                            start=(kc == 0), stop=(kc == KC - 1))
                    nc.vector.tensor_copy(
                        v_aug[:, jc, cg * 6:(cg + 1) * 6, 0:DH],
                        ps[:].rearrange("p (h d) -> p h d", d=DH))

            # ---------------- per-batch chain pieces ------------------------
            yb = ap.tile([P, KC], f32, name="yb")
            for kc in range(KC):
                nc.vector.tensor_reduce(yb[:, kc:kc + 1], yT[:, kc, :],
                                        axis=mybir.AxisListType.X, op=ALU.add)
            ybt = ap.tile([P, KC], bf16, name="ybt")
            nc.vector.tensor_scalar_mul(ybt[:], yb[:], 1.0 / S)

            def vchain_cm(vec_cm, w_t, bias_cm, name):
                # chan-major out [128, 6] f32 = vec @ W + bias, computed
                # directly in chan-major form (stationary = weight chunk,
                # moving = the [128,1] chan-major vector chunk): no DRAM
                # bounce, so the sync DMA queue never blocks on PE progress.
                out = ap.tile([P, KC], f32, name=f"{name}_cm32")
                for mc in range(KC):
                    ps = pp.tile([P, 1], f32, name=f"{name}ps{mc}",
                                 tag="sps", bufs=2)
                    for kc in range(KC):
                        nc.tensor.matmul(ps[:],
                                         w_t[:, kc, mc * P:(mc + 1) * P],
                                         vec_cm[:, kc:kc + 1],
                                         start=(kc == 0), stop=(kc == KC - 1))
                    if bias_cm is not None:
                        nc.vector.tensor_add(out[:, mc:mc + 1], ps[:],
                                             bias_cm[:, mc:mc + 1])
                    else:
                        nc.vector.tensor_copy(out[:, mc:mc + 1], ps[:])
                return out

            # ---------------- attention: pairs pipelined against K/V -------
            w_dv = wtile("dv_w")
            b_dv = btile("dv_b")
            w_do0 = wtile("diff_out_w", half=0)
            b_dout = btile("diff_out_b")
            if has_vvb:
                b_vv = btile("vv_b")
            vanT = ap.tile([P, KC, RV], bf16, name="vanT")
            acc_t2 = ap.tile([P, KC, RV], f32, name="acc_t2")

            def van_partial(kc, w_t, acc, tg):
                # acc += w[kc-chunk].T @ vanT[kc] as soon as the pair-kc
                # attention output lands; accumulate in SBUF f32 on GPSIMD.
                for mc in range(KC):
                    ps = pp.tile([P, RV], f32, name=f"{tg}p{kc}_{mc}",
                                 tag="pv", bufs=3)
                    nc.tensor.matmul(ps[:], w_t[:, kc, mc * P:(mc + 1) * P],
                                     vanT[:, kc, :], start=True, stop=True)
                    if kc == 0:
                        nc.vector.tensor_copy(acc[:, mc, :], ps[:])
                    else:
                        nc.vector.tensor_add(acc[:, mc, :], acc[:, mc, :],
                                             ps[:])

            def pair_tail(hq, pvs):
                hp = hq
                for hh in range(2):
                    invZb = lp.tile([1, RV], bf16, name=f"invZb{2 * hq + hh}",
                                    tag="invZb")
                    with nc.allow_low_precision(reason="1/Z feeds bf16 mul"):
                        nc.vector.reciprocal(invZb[:], pvs[hh][DH:DH + 1, :])
                    bc = pp.tile([64, RV], f32, name=f"bc{2 * hq + hh}",
                                 tag="big", bufs=3)
                    nc.tensor.matmul(bc[:], ones64[:], invZb[:],
                                     start=True, stop=True)
                    bcs = lp.tile([64, RV], bf16, name=f"bcs{2 * hq + hh}",
                                  tag="bcs")
                    nc.vector.tensor_copy(bcs[:], bc[:])
                    nc.vector.tensor_mul(vanT[hh * 64:hh * 64 + 64, hp, :],
                                         pvs[hh][0:DH, :], bcs[:])
                    if has_vvb:
                        nc.vector.tensor_scalar_add(
                            vanT[hh * 64:hh * 64 + 64, hp, :],
                            vanT[hh * 64:hh * 64 + 64, hp, :],
                            b_vv[hh * 64:hh * 64 + 64, hp:hp + 1])

            def pair_block(hp, prev):
                # scores+exp for pair hp, with the PV matmuls of the previous
                # pair interleaved into the same jc loop so the PE never
                # stalls on ACT's exp backlog. Both heads of the pair share
                # one [128,512] PSUM bank -> one exp ACTIVATE per block.
                e = lp.tile([P, JC, 512], bf16, name=f"e{hp}", tag="expT",
                            bufs=3)
                if prev is not None:
                    hq, eq = prev
                    pvs = [pp.tile([DH + 1, RV], f32, name=f"pv{2 * hq + hh}",
                                   tag="pv", bufs=3) for hh in range(2)]
                for jc in range(JC):
                    for hh in range(2):
                        lo = hh * 64
                        sc = pp.tile([P, RV], f32, name=f"sc{hp}_{jc}_{hh}",
                                     tag="big", bufs=3)
                        nc.tensor.matmul(
                            sc[:],
                            kT[lo:lo + 64, hp, jc * P:(jc + 1) * P],
                            qT[lo:lo + 64, hp, :],
                            start=True, stop=True)
                        nc.scalar.activation(
                            e[:, jc, hh * RV:(hh + 1) * RV], sc[:],
                            AF.Exp, scale=SCALE)
                    if prev is not None:
                        for hh in range(2):
                            nc.tensor.matmul(
                                pvs[hh][:], v_aug[:, jc, 2 * hq + hh, :],
                                eq[:, jc, hh * RV:(hh + 1) * RV],
                                start=(jc == 0), stop=(jc == JC - 1))
                if prev is not None:
                    pair_tail(hq, pvs)
                return e

            def last_pv(hq, eq):
                pvs = [pp.tile([DH + 1, RV], f32, name=f"pv{2 * hq + hh}",
                               tag="pv", bufs=3) for hh in range(2)]
                for jc in range(JC):
                    for hh in range(2):
                        nc.tensor.matmul(
                            pvs[hh][:], v_aug[:, jc, 2 * hq + hh, :],
                            eq[:, jc, hh * RV:(hh + 1) * RV],
                            start=(jc == 0), stop=(jc == JC - 1))
                pair_tail(hq, pvs)

            kproj(0)
            kproj(1)
            kproj(2)
            vproj(0)
            e0 = pair_block(0, None)
            m32 = vchain_cm(ybt, w_dv, b_dv, "m")
            m_cm = ap.tile([P, KC], bf16, name="m_cm")
            nc.vector.tensor_copy(m_cm[:], m32[:])
            e1h = pair_block(1, (0, e0))
            e2h = pair_block(2, (1, e1h))
            w_vfc = wtile("van_fc_w")
            b_vfc = btile("van_fc_b")
            van_partial(0, w_vfc, acc_t2, "t2")
            kproj(3)
            kproj(4)
            kproj(5)
            vproj(1)
            # gating-weight prefetch: issued mid-attention in need-order.
            # With bufs=6 the slot gates (6-back readers) all resolve at or
            # before each weight's emission point, so the in-order sync DMA
            # queue never blocks ahead of the z1 trigger.
            w_dth1 = wtile("d_theta_w", half=1)
            w_WV = wtile("WV_w")
            w_vg0 = wtile("v_gamma_w", half=0)
            w_vo0 = wtile("van_out_w", half=0)
            b_vo = btile("van_out_b")
            b_dfc = btile("diff_fc_b")
            ws_gate = wsp.tile([P, 2 * KC, 1], bf16, name="ws_gate", tag="ws")
            nc.sync.dma_start(ws_gate[:, :, 0], wd["gate_w"][:])
            ws_nf = wsp.tile([P, 2 * KC, 1], bf16, name="ws_nf", tag="ws")
            nc.sync.dma_start(ws_nf[:, :, 0], wd["nf_out_w"][:])
            van_partial(1, w_vfc, acc_t2, "t2")
            w_dfc = wtile("diff_fc_w")
            e3h = pair_block(3, (2, e2h))
            b_dfus = btile("diff_fus_b")
            b_vfus = btile("van_fus_b")
            b_nf = btile("nf_b")
            b_fin = btile("final_b")
            e4h = pair_block(4, (3, e3h))
            van_partial(2, w_vfc, acc_t2, "t2")
            e5h = pair_block(5, (4, e4h))
            van_partial(3, w_vfc, acc_t2, "t2")
            # van_partial(4) reads vanT[4] written by pair_tail inside the
            # pair-5 block just emitted; the bias2 vchain gives the DVE chain
            # time to land before the PE reaches it.
            bias2 = vchain_cm(m_cm, w_do0, b_dout, "bias2")
            van_partial(4, w_vfc, acc_t2, "t2")
            last_pv(5, e5h)
            van_partial(5, w_vfc, acc_t2, "t2")
            # vg1 slot-gates on van_partial(5) (vfc's last read): emit here so
            # the queue unblocks right away; it lands during the e1 gemm.
            w_vg1 = wtile("v_gamma_w", half=1)

            # ---------------- gating network --------------------------------
            def gemm(pairs, func, bias_t=None, accum_t=None, name="g",
                     out_dt=bf16, pre=None):
                out = ap.tile([P, KC, RV], out_dt, name=name)
                nmm = len(pairs) * KC
                for mc in range(KC):
                    ps = pp.tile([P, RV], f32, name=f"{name}ps{mc}", tag="big",
                                 bufs=3)
                    i = 0
                    for wt, at in pairs:
                        for kc in range(KC):
                            nc.tensor.matmul(ps[:],
                                             wt[:, kc, mc * P:(mc + 1) * P],
                                             at[:, kc, :],
                                             start=(i == 0), stop=(i == nmm - 1))
                            i += 1
                    src = ps
                    if pre is not None:
                        tmp = lp.tile([P, RV], f32, name=f"{name}pre{mc}",
                                      tag="pretmp")
                        nc.vector.tensor_add(tmp[:], ps[:], pre[:, mc, :])
                        src = tmp
                    if func == AF.Identity and accum_t is None:
                        if bias_t is not None:
                            nc.vector.tensor_scalar_add(out[:, mc, :], src[:],
                                                        bias_t[:, mc:mc + 1])
                        else:
                            nc.vector.tensor_copy(out[:, mc, :], src[:])
                    else:
                        nc.scalar.activation(
                            out[:, mc, :], src[:], func,
                            bias=(bias_t[:, mc:mc + 1] if bias_t is not None
                                  else 0.0),
                            accum_out=(accum_t[:, mc:mc + 1]
                                       if accum_t is not None else None))
                return out

            def ag_start(part, name):
                gi = dp.tile([P, KC], f32, name=f"gi_{name}")
                go = dp.tile([4 * P, KC], f32, name=f"go_{name}")
                nc.sync.dma_start(gi[:], part[:])
                nc.gpsimd.collective_compute(
                    "AllGather", ALU.bypass, replica_groups=GROUPS,
                    ins=[gi[:]], outs=[go[:]])
                return go

            def ag_finish(go, name):
                zt = ap.tile([P, 4, KC], f32, name=f"zt_{name}")
                nc.sync.dma_start(zt[:], go.rearrange("(r p) c -> p r c", p=P))
                z = ap.tile([P, KC], f32, name=f"z_{name}")
                nc.vector.tensor_add(z[:], zt[:, 0, :], zt[:, 1, :])
                nc.vector.tensor_add(z[:], z[:], zt[:, 2, :])
                nc.vector.tensor_add(z[:], z[:], zt[:, 3, :])
                return z

            theta2 = ap.tile([P, KC, RV], bf16, name="theta2")
            for mc in range(KC):
                nc.scalar.activation(theta2[:, mc, :], acc_t2[:, mc, :],
                                     AF.Tanh, bias=b_vfc[:, mc:mc + 1])

            part1 = ap.tile([P, KC], f32, name="part1")
            e1 = gemm([(w_dth1, theta2)], AF.Exp, accum_t=part1, name="e1")
            go1 = ag_start(part1, "z1")
            # readback enqueued right behind the trigger: anything between
            # them would delay the gather result by ~600ns per queue entry.
            z1 = ag_finish(go1, "z1")

            # --- AllGather-1 bubble fillers (independent of z1) -------------
            # gamma1 and voa interleaved at the mc level so each one's ACT
            # epilogues hide under the other's matmuls. Weight DMAs for the
            # post-z1 GEMMs are emitted here in need-order; each slot gate
            # resolves no later than the previous one (monotone), so the
            # queue drains without head-of-line blocking.
            w_do1 = wtile("diff_out_w", half=1)
            gamma1 = ap.tile([P, KC, RV], bf16, name="gamma1")
            voa = ap.tile([P, KC, RV], f32, name="voa")
            for mc in range(KC):
                ps1 = pp.tile([P, RV], f32, name=f"g1ps{mc}", tag="big",
                              bufs=3)
                for kc in range(KC):
                    nc.tensor.matmul(ps1[:], w_WV[:, kc, mc * P:(mc + 1) * P],
                                     vanT[:, kc, :],
                                     start=(kc == 0), stop=(kc == KC - 1))
                nc.scalar.activation(gamma1[:, mc, :], ps1[:], AF.Tanh)
                ps2 = pp.tile([P, RV], f32, name=f"voaps{mc}", tag="big",
                              bufs=3)
                for kc in range(KC):
                    nc.tensor.matmul(ps2[:], w_vo0[:, kc, mc * P:(mc + 1) * P],
                                     vanT[:, kc, :],
                                     start=(kc == 0), stop=(kc == KC - 1))
                nc.vector.tensor_scalar_add(voa[:, mc, :], ps2[:],
                                            b_vo[:, mc:mc + 1])
            w_dfus = wtile("diff_fus_w")
            z2a = gemm([(w_vg0, gamma1)], AF.Identity, name="z2a", out_dt=f32)
            w_vo1 = wtile("van_out_w", half=1)
            ps_nf = pp.tile([1, RV], f32, name="nfps", tag="sps", bufs=2)
            for kc in range(KC):
                nc.tensor.matmul(ps_nf[:], ws_nf[:, kc, :], vanT[:, kc, :],
                                 start=(kc == 0), stop=False,
                                 skip_group_check=True)
            w_vfus = wtile("van_fus_w")

            s1 = ap.tile([P, KC], f32, name="s1")
            nc.vector.reciprocal(s1[:], z1[:])
            nc.vector.tensor_mul(s1[:], s1[:], m32[:])
            dth = ap.tile([P, KC, RV], bf16, name="dth")
            for mc in range(KC):
                nc.vector.tensor_scalar_mul(dth[:, mc, :], e1[:, mc, :],
                                            s1[:, mc:mc + 1])

            gamma2 = gemm([(w_dfc, dth)], AF.Tanh, bias_t=b_dfc, name="gamma2")
            w_nf = wtile("nf_w")
            part2 = ap.tile([P, KC], f32, name="part2")
            e2 = gemm([(w_vg1, gamma2)], AF.Exp, accum_t=part2, pre=z2a,
                      name="e2")
            go2 = ag_start(part2, "z2")
            z2 = ag_finish(go2, "z2")
            w_fin = wtile("final_w")

            # --- AllGather-2 bubble fillers --------------------------------
            dout = gemm([(w_do1, dth)], AF.Tanh, bias_t=bias2, name="dout")
            dfus = gemm([(w_dfus, dout)], AF.Tanh, bias_t=b_dfus, name="dfus")

            s2 = ap.tile([P, KC], f32, name="s2")
            nc.vector.reciprocal(s2[:], z2[:])
            ag = ap.tile([P, KC, RV], bf16, name="ag")
            for mc in range(KC):
                nc.vector.scalar_tensor_tensor(
                    ag[:, mc, :], e2[:, mc, :], s2[:, mc:mc + 1],
                    vanT[:, mc, :], op0=ALU.mult, op1=ALU.mult)

            vout = gemm([(w_vo1, ag)], AF.Tanh, pre=voa, name="vout")
            vfus = gemm([(w_vfus, vout)], AF.Tanh, bias_t=b_vfus, name="vfus")
            diffv = ap.tile([P, KC, RV], bf16, name="diffv")
            for mc in range(KC):
                nc.vector.tensor_sub(diffv[:, mc, :], vfus[:, mc, :],
                                     dfus[:, mc, :])

            # gate (M=1 GEMM over both fusion tensors)
            ps_g = pp.tile([1, RV], f32, name="gateps", tag="sps", bufs=2)
            i = 0
            for at, base in [(dfus, 0), (vfus, KC)]:
                for kc in range(KC):
                    nc.tensor.matmul(ps_g[:], ws_gate[:, base + kc, :],
                                     at[:, kc, :],
                                     start=(i == 0), stop=(i == 2 * KC - 1))
                    i += 1
            gb16 = ap.tile([1, RV], bf16, name="gb16")
            nc.scalar.activation(gb16[:], ps_g[:], AF.Sigmoid)
            gbc = pp.tile([P, RV], f32, name="gbc", tag="pv", bufs=3)
            nc.tensor.matmul(gbc[:], ones128[:], gb16[:], start=True, stop=True)

            fus = ap.tile([P, KC, RV], bf16, name="fus")
            for mc in range(KC):
                t2 = lp.tile([P, RV], bf16, name=f"ft2_{mc}", tag="ft2")
                nc.vector.tensor_mul(t2[:], diffv[:, mc, :], gbc[:])
                nc.vector.tensor_add(fus[:, mc, :], t2[:], dfus[:, mc, :])

            # tnf first, so the nf sigmoid/broadcast is ready before the
            # final tanh GEMM and its fused mul+store epilogue.
            tnf = gemm([(w_nf, fus)], AF.Identity, bias_t=b_nf, name="tnf")
            for kc in range(KC):
                nc.tensor.matmul(ps_nf[:], ws_nf[:, KC + kc, :], tnf[:, kc, :],
                                 start=False, stop=(kc == KC - 1),
                                 skip_group_check=True)
            nb16 = ap.tile([1, RV], bf16, name="nb16")
            nc.scalar.activation(nb16[:], ps_nf[:], AF.Sigmoid)
            nbc = pp.tile([P, RV], f32, name="nbc", tag="pv", bufs=3)
            nc.tensor.matmul(nbc[:], ones128[:], nb16[:], start=True, stop=True)

            od = out_d.rearrange("(mc p) n -> mc p n", p=P)
            for mc in range(KC):
                ps = pp.tile([P, RV], f32, name=f"ftps{mc}", tag="big", bufs=3)
                for kc in range(KC):
                    nc.tensor.matmul(ps[:], w_fin[:, kc, mc * P:(mc + 1) * P],
                                     fus[:, kc, :],
                                     start=(kc == 0), stop=(kc == KC - 1))
                ftc = lp.tile([P, RV], bf16, name=f"ftc{mc}", tag="ftc")
                nc.scalar.activation(ftc[:], ps[:], AF.Tanh,
                                     bias=b_fin[:, mc:mc + 1])
                ot = lp.tile([P, RV], f32, name=f"ot{mc}", tag="ot", bufs=3)
                nc.vector.tensor_mul(ot[:], ftc[:], nbc[:])
                nc.sync.dma_start(od[mc], ot[:])

    nc.compile()
    return nc


_CACHE = {}


def _cm(arr2d):
    # [KC*P, N] -> partition-major [P, KC*N] (chunk kc at partition p holds
    # input row kc*P + p), flattened contiguous per partition.
    n = arr2d.shape[1]
    kc = arr2d.shape[0] // P
    return np.ascontiguousarray(
        arr2d.reshape(kc, P, n).transpose(1, 0, 2).reshape(P, kc * n))


def _prep_in_maps(inputs):
    x = np.asarray(inputs["x"], np.float32)
    y = np.asarray(inputs["y"], np.float32)
    has_vvb = bool(np.any(np.asarray(inputs["vv_b"]) != 0))

    xt = x.reshape(B * S, H).T.astype(nbf16)                  # [H, 2048]
    yts = [_cm(y[b].T.astype(nbf16)) for b in range(B)]       # [P, KC*S]

    base = {}
    for w in W768 + W1536:
        base[w] = _cm(np.asarray(inputs[w], np.float32).astype(nbf16))
    for w in ["gate_w", "nf_out_w"]:
        base[w] = _cm(np.asarray(inputs[w], np.float32).astype(nbf16))
    for b in BIAS:
        base[b] = _cm(np.asarray(inputs[b], np.float32).reshape(H, 1))
    if has_vvb:
        base["vv_b"] = _cm(np.asarray(inputs["vv_b"], np.float32).reshape(H, 1))

    in_maps = []
    for c in range(8):
        bat = c // 4
        m = dict(base)
        m["xT"] = _cm(np.ascontiguousarray(xt[:, c * RV:(c + 1) * RV]))
        m["yT"] = yts[bat]
        in_maps.append(m)
    return in_maps, has_vvb


def kernel(**inputs):
    in_maps, has_vvb = _prep_in_maps(inputs)
    if has_vvb not in _CACHE:
        _CACHE[has_vvb] = build(has_vvb)
    nc = _CACHE[has_vvb]

    res = run_bass_kernel_spmd(nc, in_maps, core_ids=list(range(8)))
    full = np.concatenate([res.results[c]["outT"] for c in range(8)], axis=1)
    return np.ascontiguousarray(full.T.reshape(B, S, H)).astype(np.float32)


if __name__ == "__main__":
    rng = np.random.default_rng(0)
    ins = {"x": rng.standard_normal((B, S, H)).astype(np.float32),
           "y": rng.standard_normal((B, S, H)).astype(np.float32)}
    for w in W768 + W1536 + ["dq_w", "dk_w", "WD_w"]:
        shp = (H, H) if w not in W1536 else (2 * H, H)
        ins[w] = (rng.standard_normal(shp) * 0.02).astype(np.float32)
    ins["gate_w"] = (rng.standard_normal((2 * H, 1)) * 0.02).astype(np.float32)
    ins["nf_out_w"] = (rng.standard_normal((2 * H, 1)) * 0.02).astype(np.float32)
    for b in BIAS + ["vv_b", "dq_b", "dk_b", "d_theta_b", "v_gamma_b"]:
        ins[b] = np.zeros(H, np.float32)
    out = kernel(**ins)
    print("out", out.shape, out.dtype, np.abs(out).mean())
